# revision 1
# baseline (speedup 1.0000x reference)
"""Trainium2 Bass kernel v2 for the 2-layer DPHGNN + hyperconv GNN stack.

Differences from the v1 baseline:
- node-side intermediates (x_init, transposed h) live in SBUF, bf16
- e2v node epilogue fused into the scatter pass (no npart DRAM round trip)
- dead h stores removed
- ReduceScatter/AllGather split into 4 pipelined chunks over a padded
  160-tile edge space (edge ownership remapped chunk-major)
- dense weights/lhsT in bf16, one-hot A matrices built in bf16 (2x DVE)
- table writes batched 4 node tiles per DMA; rse/ytab marked Shared
- deeper gather buffering, loads issued from the scalar (ACT) HWDGE queue
"""

import sys
from contextlib import ExitStack

for _p in ("/opt/trn_rl_repo",):
    if _p not in sys.path:
        sys.path.append(_p)

import numpy as np

import concourse.bass as bass
import concourse.bacc as bacc
import concourse.mybir as mybir
import concourse.tile as tile
from concourse.bass_utils import run_bass_kernel_spmd
from concourse.masks import make_identity

F32 = mybir.dt.float32
BF16 = mybir.dt.bfloat16
I16 = mybir.dt.int16
AF = mybir.ActivationFunctionType

NEG_SLOPE = 0.2
P = 128
NCORES = 8
GQ = 4          # SWDGE queues
NI = 1024       # rows per dma_gather call (hard ucode limit)
WCH = 8         # chunks per gather call / A-build batch
PSW = 2         # PSUM tiles per scatter mega-window
GB_V = 8        # gather bufs, v2e stream
GB_E = 6        # gather bufs, e2v stream
RSK = 4         # ReduceScatter/AllGather pipeline chunks
RS_LAG = 10     # tiles of lag before firing an RS chunk

N_N, N_M = 50000, 20000
NS = N_N // NCORES               # 6250 nodes per core
NT_V = (NS + P - 1) // P         # 49
NT_E = 160                       # padded edge tiles (20480 rows)
ME = NT_E * P                    # 20480
CHT = NT_E // RSK                # 40 edge tiles per RS chunk
CHROWS = CHT * P                 # 5120 global rows per chunk
OWNR = CHROWS // NCORES          # 640 rows owned per core per chunk
MS_OWN = OWNR * RSK              # 2560 owned rows per core
NT_MS = MS_OWN // P              # 20 owned tiles
OWNT = OWNR // P                 # 5 owned tiles per chunk


def _wrap_idx(flat):
    L = len(flat)
    assert L % 16 == 0
    blk = np.asarray(flat, np.int16).reshape(-1, 16).T.copy()
    return np.ascontiguousarray(np.tile(blk, (8, 1)))


def _build_stream(dst, src_idx, n_tiles, cpt):
    """Destination-sorted, per-tile 128-padded entry stream."""
    order = np.argsort(dst, kind="stable")
    dsts = np.asarray(dst)[order]
    srcs = np.asarray(src_idx)[order]
    tile_of = dsts // P
    counts = np.bincount(tile_of, minlength=n_tiles)
    base = np.concatenate([[0], np.cumsum(cpt * P)])
    L = int(base[-1])
    gidx = np.zeros(L, np.int64)
    ec = -np.ones(L, np.float32)
    starts = np.concatenate([[0], np.cumsum(counts)])
    off = np.arange(len(dsts)) - starts[tile_of]
    slot = base[tile_of] + off
    gidx[slot] = srcs
    ec[slot] = dsts - tile_of * P
    return gidx, ec


def _own_rows(c):
    """Global edge rows owned by core c (chunk-major RS layout)."""
    rows = []
    for k in range(RSK):
        r0 = k * CHROWS + c * OWNR
        rows.append(np.arange(r0, r0 + OWNR))
    return np.concatenate(rows)


def _prep(inputs):
    V = np.asarray(inputs["V"]).astype(np.int64)
    E = np.asarray(inputs["E"]).astype(np.int64)
    X = np.asarray(inputs["X"], np.float32)
    S = np.asarray(inputs["S"], np.float32)

    deg_v = np.bincount(V, minlength=N_N).astype(np.float64)
    cnt_e = np.bincount(E, minlength=N_M).astype(np.float64)
    deginv = np.where(deg_v > 0, 1.0 / np.maximum(deg_v, 1.0), 0.0)
    De = np.zeros(N_M, np.float64)
    np.add.at(De, E, deg_v[V])
    De = De / (cnt_e + 1.0)
    De_inv = np.where(De > 0, De ** -0.5, 1.0)
    coef_e = np.where(cnt_e > 0, De_inv / np.maximum(cnt_e, 1.0), 0.0)
    Dv_inv = np.where(deg_v > 0, deg_v ** -0.5, 0.0)

    owner = V // NS
    v2e_raw, e2v_raw = [], []
    for c in range(NCORES):
        m = owner == c
        Vl = V[m] - c * NS
        Ee = E[m]
        v2e_raw.append((Ee, Vl))
        e2v_raw.append((Vl, Ee))

    def caps(raw, n_tiles, min1=True):
        cpt = None
        for dst, _ in raw:
            counts = np.bincount(np.asarray(dst) // P, minlength=n_tiles)
            c1 = (counts + P - 1) // P
            if min1:
                c1 = np.maximum(1, c1)
            cpt = c1 if cpt is None else np.maximum(cpt, c1)
        return cpt

    # e2v as one stream (A = whole ytab) plus an unused empty B; the split
    # variant measured slower, so B is disabled.
    e2v_a = e2v_raw
    e2v_b = [(np.zeros(0, np.int64), np.zeros(0, np.int64))
             for _ in range(NCORES)]

    cpt_v2e = caps(v2e_raw, NT_E)
    cpt_e2v_a = caps(e2v_a, NT_V, min1=True)
    cpt_e2v_b = caps(e2v_b, NT_V, min1=False)
    Lv = int(np.sum(cpt_v2e) * P)
    LeA = int(np.sum(cpt_e2v_a) * P)
    LeB = int(np.sum(cpt_e2v_b) * P)
    LvP = ((Lv + NI - 1) // NI) * NI
    LeAP = ((LeA + NI - 1) // NI) * NI
    LeBP = max(NI, ((LeB + NI - 1) // NI) * NI)

    def pad_stream(g, ec, LP):
        gi = np.full(LP, -1, np.int64)
        gi[: len(g)] = g
        ecp = np.full(LP, -1.0, np.float32)
        ecp[: len(ec)] = ec
        ecb = ecp.astype(np.dtype("bfloat16"))
        return gi, np.ascontiguousarray(ecb.reshape(-1, P).T)

    cores = []
    for c in range(NCORES):
        gv, ecv = _build_stream(*v2e_raw[c], NT_E, cpt_v2e)
        gea, ecea = _build_stream(e2v_a[c][0], e2v_a[c][1], NT_V, cpt_e2v_a)
        geb, eceb = _build_stream(e2v_b[c][0], e2v_b[c][1], NT_V, cpt_e2v_b)
        gv_p, ecv_2d = pad_stream(gv, ecv, LvP)
        gea_p, ecea_2d = pad_stream(gea, ecea, LeAP)
        geb_p, eceb_2d = pad_stream(geb, eceb, LeBP)
        cores.append(dict(gv_p=gv_p, ecv_2d=ecv_2d,
                          gea_p=gea_p, ecea_2d=ecea_2d,
                          geb_p=geb_p, eceb_2d=eceb_2d))

    def regs(L, LP):
        return [int(max(0, min(L - k * NI, NI))) for k in range(LP // NI)]

    regs_v = regs(Lv, LvP)
    regs_ea = regs(LeA, LeAP)
    regs_eb = regs(LeB, LeBP)

    bf = np.dtype("bfloat16")
    g = lambda k: np.asarray(inputs[k], np.float32)
    W = {}
    for l in range(2):
        Wv, bv, a = g(f"Wv{l}"), g(f"bv{l}"), g(f"a{l}")
        Wx, bx = g(f"Wx{l}"), g(f"bx{l}")
        Wt, bt = g(f"Wt{l}"), g(f"bt{l}")
        Wva = np.concatenate([Wv, (Wv @ a)[:, None]], axis=1)
        bva = np.concatenate([bv, [float(bv @ a)]])
        Wt_top, Wt_bot = Wt[:256], Wt[256:]
        btf = bt - Wt_top.sum(axis=0)
        nh = Wva.shape[0] // P
        for hi in range(nh):
            W[f"Wva{l}h{hi}"] = np.ascontiguousarray(
                Wva[hi * P:(hi + 1) * P]).astype(bf)
            W[f"Wx{l}h{hi}"] = np.ascontiguousarray(
                Wx[hi * P:(hi + 1) * P]).astype(bf)
        W[f"bva{l}"] = np.tile(bva[None, :].astype(np.float32), (P, 1))
        W[f"bx{l}"] = np.tile((bx - 1.0)[None, :], (P, 1))
        W[f"Wt{l}h0"] = np.ascontiguousarray(Wt_top[:128]).astype(bf)
        W[f"Wt{l}h1"] = np.ascontiguousarray(Wt_top[128:]).astype(bf)
        W[f"Wt{l}bot"] = np.ascontiguousarray(Wt_bot).astype(bf)
        W[f"bt{l}"] = np.tile(btf[None, :].astype(np.float32), (P, 1))
    Wf = g("Wf")
    W["Wfh0"] = np.ascontiguousarray(Wf[:128]).astype(bf)
    W["Wfh1"] = np.ascontiguousarray(Wf[128:]).astype(bf)
    W["bf"] = np.tile(g("bf")[None, :], (P, 1))

    iota = np.tile(np.arange(P, dtype=np.float32)[None, :], (P, 1))
    iota_rep = np.ascontiguousarray(
        np.broadcast_to(iota[:, None, :], (P, WCH, P))).astype(bf)

    def cols(arr, n_tiles):
        out = np.zeros((P, n_tiles), np.float32)
        a = np.asarray(arr, np.float32)
        for t in range(n_tiles):
            seg = a[t * P:(t + 1) * P]
            out[: len(seg), t] = seg
        return out

    in_maps = []
    for c in range(NCORES):
        d = cores[c]
        own = _own_rows(c)
        own_real = own[own < N_M]
        ST_own = np.zeros((MS_OWN, 64), np.float32)
        ST_own[own < N_M] = S[own_real]
        coef_own = np.zeros(MS_OWN, np.float32)
        coef_own[own < N_M] = coef_e[own_real]
        im = dict(
            XT=np.ascontiguousarray(X[c * NS:(c + 1) * NS].T).astype(bf),
            ST=np.ascontiguousarray(ST_own.T).astype(bf),
            gv_idx=_wrap_idx(d["gv_p"]),
            gea_idx=_wrap_idx(d["gea_p"]), geb_idx=_wrap_idx(d["geb_p"]),
            ec_v=d["ecv_2d"], ec_ea=d["ecea_2d"], ec_eb=d["eceb_2d"],
            iota_rep=iota_rep,
            deginv_c=cols(deginv[c * NS:(c + 1) * NS], NT_V),
            dvinv_c=cols(Dv_inv[c * NS:(c + 1) * NS], NT_V),
            coef_c=cols(coef_own, NT_MS),
        )
        im.update(W)
        in_maps.append(im)

    meta = dict(cpt_v2e=[int(x) for x in cpt_v2e],
                cpt_e2v_a=[int(x) for x in cpt_e2v_a],
                cpt_e2v_b=[int(x) for x in cpt_e2v_b],
                LvP=LvP, LeAP=LeAP, LeBP=LeBP,
                regs_v=regs_v, regs_ea=regs_ea, regs_eb=regs_eb)
    return in_maps, meta


# ---------------------------------------------------------------------------

def build_program(meta):
    ESV, ESE, ESH = 384, 256, 128

    nc = bacc.Bacc("TRN2", target_bir_lowering=False, debug=False,
                   num_devices=NCORES, num_swdge_queues=GQ)

    def din(name, shape, dt=F32):
        return nc.dram_tensor(name, shape, dt, kind="ExternalInput")

    XT = din("XT", [P, NS], BF16)
    ST = din("ST", [64, MS_OWN], BF16)
    gv_idx = din("gv_idx", [P, meta["LvP"] // 16], I16)
    gea_idx = din("gea_idx", [P, meta["LeAP"] // 16], I16)
    geb_idx = din("geb_idx", [P, meta["LeBP"] // 16], I16)
    nch_v = meta["LvP"] // P
    nch_ea = meta["LeAP"] // P
    nch_eb = meta["LeBP"] // P
    ec_v = din("ec_v", [P, nch_v], BF16)
    ec_ea = din("ec_ea", [P, nch_ea], BF16)
    ec_eb = din("ec_eb", [P, nch_eb], BF16)
    iota_rep = din("iota_rep", [P, WCH, P], BF16)
    deginv_c = din("deginv_c", [P, NT_V])
    dvinv_c = din("dvinv_c", [P, NT_V])
    coef_c = din("coef_c", [P, NT_MS])
    wnames = (["Wva0h0", "Wx0h0", "Wva1h0", "Wva1h1", "Wx1h0", "Wx1h1",
               "Wt0h0", "Wt0h1", "Wt1h0", "Wt1h1", "Wfh0", "Wfh1",
               "Wt0bot", "Wt1bot"],
              ["bva0", "bx0", "bva1", "bx1", "bt0", "bt1", "bf"])
    wshapes = dict(Wva0h0=[P, 257], Wx0h0=[P, 256],
                   Wva1h0=[P, 257], Wva1h1=[P, 257],
                   Wx1h0=[P, 256], Wx1h1=[P, 256],
                   Wt0h0=[P, 256], Wt0h1=[P, 256],
                   Wt1h0=[P, 256], Wt1h1=[P, 256],
                   Wfh0=[P, 128], Wfh1=[P, 128],
                   Wt0bot=[64, 256], Wt1bot=[64, 256],
                   bva0=[P, 257], bx0=[P, 256], bva1=[P, 257], bx1=[P, 256],
                   bt0=[P, 256], bt1=[P, 256], bf=[P, 128])
    Wd = {k: din(k, wshapes[k], BF16) for k in wnames[0]}
    Wd.update({k: din(k, wshapes[k], F32) for k in wnames[1]})

    yout = nc.dram_tensor("yout", [NS, 128], F32, kind="ExternalOutput")

    rg = [list(range(NCORES))]

    with tile.TileContext(nc) as tc:
        ctx = ExitStack()
        sbuf = ctx.enter_context(tc.tile_pool(name="sbuf", bufs=2))
        psum = ctx.enter_context(tc.tile_pool(name="psum", bufs=2, space="PSUM"))
        dram = ctx.enter_context(tc.tile_pool(name="dram", bufs=1, space="DRAM"))
        cons = ctx.enter_context(tc.tile_pool(name="cons", bufs=1))

        iota_t = cons.tile([P, WCH, P], BF16, name="iota_t")
        nc.scalar.dma_start(iota_t[:], iota_rep[:])
        ident = cons.tile([P, P], F32, name="ident")
        make_identity(nc, ident[:])
        wt = {}
        for k, h in Wd.items():
            t = cons.tile(list(h.shape), h.dtype, name=f"w_{k}")
            nc.scalar.dma_start(t[:], h[:])
            wt[k] = t
        st_t = cons.tile([64, MS_OWN], BF16, name="st_t")
        nc.sync.dma_start(st_t[:], ST[:])
        ecv_t = cons.tile([P, nch_v], BF16, name="ecv_t")
        nc.scalar.dma_start(ecv_t[:], ec_v[:])
        ecea_t = cons.tile([P, nch_ea], BF16, name="ecea_t")
        nc.scalar.dma_start(ecea_t[:], ec_ea[:])
        eceb_t = cons.tile([P, nch_eb], BF16, name="eceb_t")
        nc.scalar.dma_start(eceb_t[:], ec_eb[:])
        gvi_t = cons.tile([P, meta["LvP"] // 16], I16, name="gvi_t")
        nc.sync.dma_start(gvi_t[:], gv_idx[:])
        geai_t = cons.tile([P, meta["LeAP"] // 16], I16, name="geai_t")
        nc.sync.dma_start(geai_t[:], gea_idx[:])
        gebi_t = cons.tile([P, meta["LeBP"] // 16], I16, name="gebi_t")
        nc.sync.dma_start(gebi_t[:], geb_idx[:])
        dgi_t = cons.tile([P, NT_V], F32, name="dgi_t")
        nc.scalar.dma_start(dgi_t[:], deginv_c[:])
        dvi_t = cons.tile([P, NT_V], F32, name="dvi_t")
        nc.scalar.dma_start(dvi_t[:], dvinv_c[:])
        cf_t = cons.tile([P, NT_MS], F32, name="cf_t")
        nc.scalar.dma_start(cf_t[:], coef_c[:])
        xt_t = cons.tile([P, NS], BF16, name="xt_t")
        nc.sync.dma_start(xt_t[:], XT[:])

        # SBUF-resident node-side state (bf16)
        xinit_sb = cons.tile([P, NT_V, 256], BF16, name="xinit_sb")
        hT = [cons.tile([P, NT_V * P], BF16, name=f"hT{hi}") for hi in range(2)]

        qctr = [0]

        def scatter_pass(streams, used_cols, n_tiles, on_tile):
            """Gather + one-hot-matmul segment sum over dest tiles.

            streams: list of dicts (in_ap, es, idx_t, ec_t, cpt, regs, tag,
            gb); chunks are consumed tile-major, streams in order within a
            tile.  on_tile(t, psum_ap, q, last_in_window, n_in_window) fires
            when tile t's accumulation is complete."""
            S = len(streams)
            chunk_lists = []
            for st in streams:
                tof = []
                for t, n in enumerate(st["cpt"]):
                    tof += [t] * n
                chunk_lists.append(tof)
            order = []
            ks = [0] * S
            for t in range(n_tiles):
                for s in range(S):
                    for _ in range(streams[s]["cpt"][t]):
                        order.append((s, ks[s]))
                        ks[s] += 1
            first_c, last_c = {}, {}
            for pos, (s, k) in enumerate(order):
                t = chunk_lists[s][k]
                first_c.setdefault(t, pos)
                last_c[t] = pos
            g_tiles = [[None] * len(st["regs"]) for st in streams]
            emitted = [0] * S

            def ensure_emitted(s, upto):
                st = streams[s]
                while emitted[s] <= min(upto, len(st["regs"]) - 1):
                    call = emitted[s]
                    if st["regs"][call] > 0:
                        gt = sbuf.tile([P, WCH, st["es"]], BF16,
                                       tag=st["ring"], bufs=st["gb"],
                                       name=f"g{st['tag']}_{call}")
                        nc.gpsimd.dma_gather(
                            out_ap=gt[:], in_ap=st["in_ap"],
                            idxs_ap=st["idx_t"][:, call * (NI // 16):
                                                (call + 1) * (NI // 16)],
                            num_idxs=NI, num_idxs_reg=st["regs"][call],
                            elem_size=st["es"], queue_num=qctr[0] % GQ)
                        qctr[0] += 1
                        g_tiles[s][call] = gt
                    emitted[s] += 1

            a_cur = [[None, -1] for _ in range(S)]
            mega = [None, -1]
            for pos, (s, k) in enumerate(order):
                st = streams[s]
                t = chunk_lists[s][k]
                call, j = k // WCH, k % WCH
                ensure_emitted(s, call + st["gb"] - 1)
                gt = g_tiles[s][call]
                if gt is None:
                    continue
                w = k // WCH
                if a_cur[s][1] != w:
                    ab = sbuf.tile([P, WCH, P], BF16, tag=f"A{s}", bufs=2,
                                   name=f"A{st['tag']}_{w}")
                    nc.vector.tensor_tensor(
                        out=ab[:],
                        in0=st["ec_t"][:, w * WCH:(w + 1) * WCH].to_broadcast(
                            [P, WCH, P]),
                        in1=iota_t[:],
                        op=mybir.AluOpType.is_equal)
                    a_cur[s] = [ab, w]
                mw = t // PSW
                if mega[1] != mw:
                    mega = [psum.tile([P, PSW, 512], F32, tag="ps", bufs=2,
                                      name=f"ps{st['tag']}_{mw}"), mw]
                pt = mega[0]
                q = t % PSW
                nc.tensor.matmul(
                    out=pt[:, q, 0:used_cols],
                    lhsT=a_cur[s][0][:, j, :],
                    rhs=gt[:, j, 0:used_cols],
                    start=(pos == first_c[t]), stop=(pos == last_c[t]))
                if pos == last_c[t]:
                    last_in_w = (t % PSW == PSW - 1) or (t == n_tiles - 1)
                    on_tile(t, pt, q, last_in_w, q + 1)

        def cc_emit(kind, op, ins, outs):
            # emit collectives from the (mostly idle) scalar engine so their
            # waits never stall the gather stream on GpSimd
            bass.BassGpSimd.collective_compute(
                nc.gpsimd, kind, op, replica_groups=rg, ins=ins, outs=outs)

        def elu_u(z_ap, w, cols, tag, i):
            """relu(z) + exp(min(z,0)) = elu(z) + 1."""
            mn = sbuf.tile([P, cols], F32, tag="mn", bufs=2, name=f"mn{tag}{i}")
            nc.vector.tensor_scalar_min(out=mn[:w], in0=z_ap, scalar1=0.0)
            ex = sbuf.tile([P, cols], F32, tag="ex", bufs=2, name=f"ex{tag}{i}")
            nc.scalar.activation(ex[:w], mn[:w], AF.Exp)
            rl = sbuf.tile([P, cols], F32, tag="rl", bufs=2, name=f"rl{tag}{i}")
            nc.vector.tensor_scalar_max(out=rl[:w], in0=z_ap, scalar1=0.0)
            u = sbuf.tile([P, cols], F32, tag="u", bufs=2, name=f"u{tag}{i}")
            nc.vector.tensor_add(u[:w], rl[:w], ex[:w])
            return u

        def dense_and_table(l, lhsT_of, table, with_score):
            """Per node tile: table row block + x_init slice (SBUF)."""
            nh = 1 if l == 0 else 2
            stg4 = [None]
            tcols = 257 if with_score else 128
            for t in range(NT_V):
                w = min(P, NS - t * P)
                halves = lhsT_of(t, w)
                pf = psum.tile([P, 512], F32, tag="pd", bufs=2, name=f"pf{l}_{t}")
                if with_score:
                    for hi in range(nh):
                        nc.tensor.matmul(out=pf[:w, 0:257], lhsT=halves[hi],
                                         rhs=wt[f"Wva{l}h{hi}"][:],
                                         start=(hi == 0), stop=(hi == nh - 1))
                else:
                    for hi in range(nh):
                        nc.tensor.matmul(out=pf[:w, 0:128], lhsT=halves[hi],
                                         rhs=wt[f"Wfh{hi}"][:],
                                         start=(hi == 0), stop=(hi == nh - 1))
                if stg4[0] is None:
                    stg4[0] = sbuf.tile([P, 4, tcols], BF16, tag="stg4", bufs=2,
                                        name=f"stg4{l}_{t}")
                j4 = t % 4
                if with_score:
                    F = sbuf.tile([P, 257], F32, tag="F", bufs=2,
                                  name=f"F{l}_{t}")
                    nc.vector.tensor_add(F[:w], pf[:w, 0:257], wt[f"bva{l}"][:w])
                    lr = sbuf.tile([P, 1], F32, tag="lr", bufs=2,
                                   name=f"lr{l}_{t}")
                    nc.vector.tensor_scalar_mul(out=lr[:w],
                                                in0=F[:w, 256:257],
                                                scalar1=NEG_SLOPE)
                    ew = sbuf.tile([P, 1], F32, tag="ew", bufs=2,
                                   name=f"ew{l}_{t}")
                    nc.vector.tensor_tensor(out=ew[:w], in0=F[:w, 256:257],
                                            in1=lr[:w],
                                            op=mybir.AluOpType.max)
                    nc.scalar.activation(ew[:w], ew[:w], AF.Exp)
                    nc.scalar.activation(stg4[0][:w, j4, 0:256],
                                         F[:w, 0:256], AF.Copy,
                                         scale=ew[:w, :])
                    nc.vector.tensor_copy(out=stg4[0][:w, j4, 256:257],
                                          in_=ew[:w, :])
                else:
                    nc.vector.tensor_add(stg4[0][:w, j4, 0:128],
                                         pf[:w, 0:128], wt["bf"][:w])
                if t % 4 == 3 or t == NT_V - 1:
                    nj = j4 + 1
                    r0 = (t - nj + 1) * P
                    nc.sync.dma_start(
                        out=table[r0:r0 + nj * P, 0:tcols].rearrange(
                            "(j p) c -> p j c", p=P),
                        in_=stg4[0][:, 0:nj, :])
                    stg4[0] = None
                if with_score:
                    pi = psum.tile([P, 512], F32, tag="pd", bufs=2,
                                   name=f"pi{l}_{t}")
                    for hi in range(nh):
                        nc.tensor.matmul(out=pi[:w, 0:256], lhsT=halves[hi],
                                         rhs=wt[f"Wx{l}h{hi}"][:],
                                         start=(hi == 0), stop=(hi == nh - 1))
                    nc.vector.tensor_add(xinit_sb[:w, t, :], pi[:w, 0:256],
                                         wt[f"bx{l}"][:w])

        def edge_epilogue_tile(l, g, rse, yin):
            """Process owned tile g (rows g*128..+128 of the rse shard)."""
            if True:
                r0 = g * P
                rt = sbuf.tile([P, 257], BF16, tag="rt", bufs=2,
                               name=f"rt{l}_{g}")
                nc.scalar.dma_start(rt[:], rse[r0:r0 + P, :])
                dc = sbuf.tile([P, 1], F32, tag="dc", bufs=2,
                               name=f"dc{l}_{g}")
                nc.vector.tensor_scalar_max(out=dc[:], in0=rt[:, 256:257],
                                            scalar1=1e-35)
                di = sbuf.tile([P, 1], F32, tag="di", bufs=2,
                               name=f"di{l}_{g}")
                nc.vector.reciprocal(di[:], dc[:])
                z = sbuf.tile([P, 256], F32, tag="z", bufs=2,
                              name=f"z{l}_{g}")
                nc.scalar.activation(z[:], rt[:, 0:256], AF.Copy,
                                     scale=di[:, :])
                u = elu_u(z[:], P, 256, f"ee{l}", g)
                uT = []
                for hi in range(2):
                    pT = psum.tile([P, P], F32, tag="pT", bufs=2,
                                   name=f"pT{l}_{g}_{hi}")
                    nc.tensor.transpose(out=pT[:, 0:P],
                                        in_=u[:, hi * P:(hi + 1) * P],
                                        identity=ident[:, :])
                    sT = sbuf.tile([P, P], BF16, tag="sT", bufs=2,
                                   name=f"sT{l}_{g}_{hi}")
                    nc.vector.tensor_copy(out=sT[:], in_=pT[:])
                    uT.append(sT)
                py = psum.tile([P, 512], F32, tag="pd", bufs=2,
                               name=f"py{l}_{g}")
                nc.tensor.matmul(out=py[:, 0:256],
                                 lhsT=st_t[:, g * P:(g + 1) * P],
                                 rhs=wt[f"Wt{l}bot"][:], start=True, stop=False)
                nc.tensor.matmul(out=py[:, 0:256], lhsT=uT[0][:],
                                 rhs=wt[f"Wt{l}h0"][:], start=False, stop=False)
                nc.tensor.matmul(out=py[:, 0:256], lhsT=uT[1][:],
                                 rhs=wt[f"Wt{l}h1"][:], start=False, stop=True)
                yt = sbuf.tile([P, 256], BF16, tag="yt", bufs=2,
                               name=f"yt{l}_{g}")
                nc.vector.tensor_add(yt[:], py[:, 0:256], wt[f"bt{l}"][:])
                nc.sync.dma_start(out=yin[r0:r0 + P, :], in_=yt[:])

        def dphgnn(l, lhsT_of):
            table = dram.tile([NT_V * P, ESV], BF16, name=f"T{l}")
            dense_and_table(l, lhsT_of, table, True)

            part = dram.tile([ME, 257], BF16, name=f"part{l}")
            rse = dram.tile([MS_OWN, 257], BF16, name=f"rse{l}")
            yin = dram.tile([MS_OWN, 256], BF16, name=f"yin{l}")
            ytab = dram.tile([ME, ESE], BF16, name=f"ytab{l}")
            pstg = [None]
            rs_done = [0]

            def v2e_tile(t, pt, q, last_in_w, n_in_w):
                if pstg[0] is None:
                    pstg[0] = sbuf.tile([P, PSW, 257], BF16, tag="pstg",
                                        bufs=4, name=f"pstg{l}_{t}")
                nc.vector.tensor_copy(out=pstg[0][:, q, :],
                                      in_=pt[:, q, 0:257])
                if last_in_w:
                    rows0 = (t - n_in_w + 1) * P
                    nc.sync.dma_start(
                        out=part[rows0:rows0 + n_in_w * P, :].rearrange(
                            "(j p) c -> p j c", p=P),
                        in_=pstg[0][:, 0:n_in_w, :])
                    pstg[0] = None
                if last_in_w:
                    while (rs_done[0] < RSK
                           and t >= (rs_done[0] + 1) * CHT - 1 + RS_LAG):
                        kc = rs_done[0]
                        cc_emit("ReduceScatter", mybir.AluOpType.add,
                                [part[kc * CHROWS:(kc + 1) * CHROWS, :]],
                                [rse[kc * OWNR:(kc + 1) * OWNR, :]])
                        rs_done[0] += 1

            scatter_pass([dict(in_ap=table[:], es=ESV, idx_t=gvi_t,
                               ec_t=ecv_t, cpt=meta["cpt_v2e"],
                               regs=meta["regs_v"], tag=f"v{l}", gb=GB_V, ring="gv")],
                         257, NT_E, v2e_tile)
            while rs_done[0] < RSK:
                kc = rs_done[0]
                cc_emit("ReduceScatter", mybir.AluOpType.add,
                        [part[kc * CHROWS:(kc + 1) * CHROWS, :]],
                        [rse[kc * OWNR:(kc + 1) * OWNR, :]])
                rs_done[0] += 1

            ag_done = [0]
            for g in range(NT_MS):
                edge_epilogue_tile(l, g, rse, yin)
                while (ag_done[0] < RSK
                       and (g + 1) * P >= (ag_done[0] + 1) * OWNR):
                    k = ag_done[0]
                    cc_emit("AllGather", mybir.AluOpType.bypass,
                            [yin[k * OWNR:(k + 1) * OWNR, :]],
                            [ytab[k * CHROWS:(k + 1) * CHROWS, :]])
                    ag_done[0] += 1

            def e2v_tile(t, pt, q, last_in_w, n_in_w):
                w = min(P, NS - t * P)
                z = sbuf.tile([P, 256], F32, tag="nz", bufs=2,
                              name=f"nz{l}_{t}")
                nc.scalar.activation(z[:w], pt[:w, q, 0:256], AF.Copy,
                                     scale=dgi_t[:w, t:t + 1])
                u = elu_u(z[:w], w, 256, f"ne{l}", t)
                h = sbuf.tile([P, 256], F32, tag="h", bufs=2, name=f"h{l}_{t}")
                nc.vector.tensor_add(h[:w], u[:w], xinit_sb[:w, t, :])
                for hi in range(2):
                    pT = psum.tile([P, P], F32, tag="pT", bufs=2,
                                   name=f"hpT{l}_{t}_{hi}")
                    nc.tensor.transpose(out=pT[:, 0:w],
                                        in_=h[:w, hi * P:(hi + 1) * P],
                                        identity=ident[:w, :w])
                    nc.vector.tensor_copy(
                        out=hT[hi][:, t * P:t * P + w], in_=pT[:, 0:w])

            scatter_pass(
                [dict(in_ap=ytab[:], es=ESE, idx_t=geai_t,
                      ec_t=ecea_t, cpt=meta["cpt_e2v_a"],
                      regs=meta["regs_ea"], tag=f"ea{l}", gb=GB_E, ring="gea")],
                256, NT_V, e2v_tile)

        # layer 0
        def l0_of(t, w):
            return [xt_t[:, t * P:t * P + w]]

        dphgnn(0, l0_of)

        # layer 1
        def l1_of(t, w):
            return [hT[0][:, t * P:t * P + w], hT[1][:, t * P:t * P + w]]

        dphgnn(1, l1_of)

        # hyperconv
        table2 = dram.tile([NT_V * P, ESH], BF16, name="T2")
        dense_and_table(2, l1_of, table2, False)

        part3 = dram.tile([ME, 128], BF16, name="part3")
        rse3 = dram.tile([MS_OWN, 128], BF16, name="rse3")
        yin3 = dram.tile([MS_OWN, 128], BF16, name="yin3")
        ytab3 = dram.tile([ME, 128], BF16, name="ytab3")
        pstg3 = [None]
        rs3_done = [0]

        def v2e3_tile(t, pt, q, last_in_w, n_in_w):
            if pstg3[0] is None:
                pstg3[0] = sbuf.tile([P, PSW, 128], BF16, tag="pstg",
                                     bufs=4, name=f"pstg3_{t}")
            nc.vector.tensor_copy(out=pstg3[0][:, q, :], in_=pt[:, q, 0:128])
            if last_in_w:
                rows0 = (t - n_in_w + 1) * P
                nc.sync.dma_start(
                    out=part3[rows0:rows0 + n_in_w * P, :].rearrange(
                        "(j p) c -> p j c", p=P),
                    in_=pstg3[0][:, 0:n_in_w, :])
                pstg3[0] = None
            if last_in_w:
                while (rs3_done[0] < RSK
                       and t >= (rs3_done[0] + 1) * CHT - 1 + RS_LAG):
                    kc = rs3_done[0]
                    cc_emit("ReduceScatter", mybir.AluOpType.add,
                            [part3[kc * CHROWS:(kc + 1) * CHROWS, :]],
                            [rse3[kc * OWNR:(kc + 1) * OWNR, :]])
                    rs3_done[0] += 1

        scatter_pass([dict(in_ap=table2[:], es=ESH, idx_t=gvi_t,
                           ec_t=ecv_t, cpt=meta["cpt_v2e"],
                           regs=meta["regs_v"], tag="v2", gb=GB_V, ring="gv")],
                     128, NT_E, v2e3_tile)
        while rs3_done[0] < RSK:
            kc = rs3_done[0]
            cc_emit("ReduceScatter", mybir.AluOpType.add,
                    [part3[kc * CHROWS:(kc + 1) * CHROWS, :]],
                    [rse3[kc * OWNR:(kc + 1) * OWNR, :]])
            rs3_done[0] += 1

        ag3_done = [0]
        for g in range(NT_MS):
            r0 = g * P
            rt = sbuf.tile([P, 128], BF16, tag="rt3", bufs=2,
                           name=f"rt3_{g}")
            nc.scalar.dma_start(rt[:], rse3[r0:r0 + P, :])
            yt = sbuf.tile([P, 128], BF16, tag="yt3", bufs=2,
                           name=f"yt3_{g}")
            nc.vector.tensor_scalar_mul(out=yt[:], in0=rt[:],
                                        scalar1=cf_t[:, g:g + 1])
            nc.sync.dma_start(out=yin3[r0:r0 + P, :], in_=yt[:])
            while (ag3_done[0] < RSK
                   and (g + 1) * P >= (ag3_done[0] + 1) * OWNR):
                k = ag3_done[0]
                cc_emit("AllGather", mybir.AluOpType.bypass,
                        [yin3[k * OWNR:(k + 1) * OWNR, :]],
                        [ytab3[k * CHROWS:(k + 1) * CHROWS, :]])
                ag3_done[0] += 1

        def e2v3_tile(t, pt, q, last_in_w, n_in_w):
            w = min(P, NS - t * P)
            ot = sbuf.tile([P, 128], F32, tag="fo", bufs=2, name=f"fo_{t}")
            nc.vector.tensor_scalar_mul(out=ot[:w], in0=pt[:w, q, 0:128],
                                        scalar1=dvi_t[:w, t:t + 1])
            nc.sync.dma_start(out=yout[t * P:t * P + w, :], in_=ot[:w])

        scatter_pass(
            [dict(in_ap=ytab3[:], es=ESH, idx_t=geai_t,
                  ec_t=ecea_t, cpt=meta["cpt_e2v_a"],
                  regs=meta["regs_ea"], tag="e3a", gb=GB_E, ring="gea")],
            128, NT_V, e2v3_tile)
        ctx.close()

    nc.compile()
    return nc


_CACHED = {}


def kernel(**inputs):
    in_maps, meta = _prep(inputs)
    key = (meta["LvP"], meta["LeAP"], meta["LeBP"], tuple(meta["cpt_v2e"]),
           tuple(meta["cpt_e2v_a"]), tuple(meta["cpt_e2v_b"]))
    if key not in _CACHED:
        _CACHED[key] = build_program(meta)
    nc = _CACHED[key]
    res = run_bass_kernel_spmd(nc, in_maps, list(range(NCORES)))
    out = np.concatenate([res.results[c]["yout"] for c in range(NCORES)],
                         axis=0)
    return np.ascontiguousarray(out.astype(np.float32))



# revision 35
# speedup vs baseline: 1.6249x; 1.6249x over previous
"""Trainium2 Bass kernel v3 for the 2-layer DPHGNN + hyperconv GNN stack.

Architecture (vs v2 baseline):
- v2e is EDGE-sharded: each core owns 2500 edges (20 tiles) and scatter-sums
  only into its owned edge rows -> no partial table over the full edge space
  and NO ReduceScatter anywhere.
- Node tables are AllGather'ed instead: each core builds the premultiplied
  table rows for its local nodes; chunked AllGathers (pipelined behind the
  e2v pass) replicate them.  Layer 0 needs no table collective at all: X is
  a kernel input, so every core builds the full table locally.
- Layer-0 table uses the low-rank trick: rows are [ew*X | ew] (129 cols,
  512B gather rows instead of 768B); Wv0 is applied post-aggregation.
- Edge epilogue is fused into the v2e scatter (PSUM -> Y tile directly);
  per-layer Y tables are AllGather'ed in 4 chunks as owned tiles complete.
- Hyperconv aggregates T2 = h2@Wf+bf rows (256B) and applies coef post-sum.
- Gather indices into the 50176-row node tables exceed int16, so v2e uses
  two streams (idx < 32768 and the rest, rebased).
"""

import sys
from contextlib import ExitStack

for _p in ("/opt/trn_rl_repo",):
    if _p not in sys.path:
        sys.path.append(_p)

import numpy as np

import concourse.bass as bass
import concourse.bacc as bacc
import concourse.mybir as mybir
import concourse.tile as tile
from concourse.bass_utils import run_bass_kernel_spmd
from concourse.masks import make_identity

F32 = mybir.dt.float32
BF16 = mybir.dt.bfloat16
I16 = mybir.dt.int16
AF = mybir.ActivationFunctionType

NEG_SLOPE = 0.2
P = 128
NCORES = 8
GQ = 4          # SWDGE queues
NI = 1024       # rows per dma_gather call (hard ucode limit)
WCH = 8         # chunks per gather call / A-build batch
PSW = 2         # PSUM tiles per scatter mega-window
GB_A = 5        # gather bufs, v2e stream A
GB_B = 3        # gather bufs, v2e stream B
GB_E = 6        # gather bufs, e2v stream

N_N, N_M = 50000, 20000
NS = N_N // NCORES               # 6250 nodes per core
NT_V = 49                        # local node tiles
NSP = NT_V * P                   # 6272 padded local nodes
NGP = NCORES * NSP               # 50176 global padded nodes
ES_OWN = N_M // NCORES           # 2500 edges per core
NT_EO = 20                       # owned edge tiles
MS_OWN = NT_EO * P               # 2560 padded owned edge rows
ME = NCORES * MS_OWN             # 20480 global padded edges
K_Y = 4                          # ytab AG chunks per layer
OWNR = MS_OWN // K_Y             # 640 local rows per ytab chunk
CH_E = ME // K_Y                 # 5120 global rows per ytab chunk
K_T = 7                          # table write groups (7 tiles each)
TCH_L = NSP // K_T               # 896 local rows per write group
TA_L = 4 * TCH_L                 # 3584 local rows in table half A
TB_L = NSP - TA_L                # 2688 local rows in table half B
SPLIT = NCORES * TA_L            # 28672 global rows in half A (< int16 max)
NGB = NCORES * TB_L              # 21504 global rows in half B
SHARED_AG = True                 # AllGather outputs in Shared address space


def _gid_edge(e):
    c = e // ES_OWN
    r = e - c * ES_OWN
    k = r // OWNR
    return k * CH_E + c * OWNR + (r - k * OWNR)


def _tgid_node(v):
    """Node-table row layout: two AllGather halves, block-major within each.

    half A = local rows [0, 3584) of each core -> global [c*3584 + u]
    half B = local rows [3584, 6272)          -> global SPLIT + [c*2688 + u']
    """
    c = v // NS
    u = v - c * NS
    return np.where(u < TA_L, c * TA_L + u, SPLIT + c * TB_L + (u - TA_L))


def _wrap_idx(flat):
    L = len(flat)
    assert L % 16 == 0
    blk = np.asarray(flat, np.int16).reshape(-1, 16).T.copy()
    return np.ascontiguousarray(np.tile(blk, (8, 1)))


def _build_stream(dst, src_idx, n_tiles, cpt):
    """Destination-sorted, per-tile 128-padded entry stream."""
    order = np.argsort(dst, kind="stable")
    dsts = np.asarray(dst)[order]
    srcs = np.asarray(src_idx)[order]
    tile_of = dsts // P
    counts = np.bincount(tile_of, minlength=n_tiles)
    base = np.concatenate([[0], np.cumsum(cpt * P)])
    L = int(base[-1])
    gidx = np.zeros(L, np.int64)
    ec = -np.ones(L, np.float32)
    starts = np.concatenate([[0], np.cumsum(counts)])
    off = np.arange(len(dsts)) - starts[tile_of]
    slot = base[tile_of] + off
    gidx[slot] = srcs
    ec[slot] = dsts - tile_of * P
    return gidx, ec


def _caps(raw, n_tiles, min1):
    cpt = None
    for dst, _ in raw:
        counts = np.bincount(np.asarray(dst) // P, minlength=n_tiles)
        c1 = (counts + P - 1) // P
        if min1:
            c1 = np.maximum(1, c1)
        cpt = c1 if cpt is None else np.maximum(cpt, c1)
    return cpt


def _pad_stream(g, ec, LP):
    gi = np.full(LP, -1, np.int64)
    gi[: len(g)] = g
    ecp = np.full(LP, -1.0, np.float32)
    ecp[: len(ec)] = ec
    ecb = ecp.astype(np.dtype("bfloat16"))
    return gi, np.ascontiguousarray(ecb.reshape(-1, P).T)


def _regs(L, LP):
    return [int(max(0, min(L - k * NI, NI))) for k in range(LP // NI)]


def _cols(arr, n_tiles):
    out = np.zeros((P, n_tiles), np.float32)
    a = np.asarray(arr, np.float32)
    for t in range(n_tiles):
        seg = a[t * P:(t + 1) * P]
        out[: len(seg), t] = seg
    return out


def _prep(inputs):
    V = np.asarray(inputs["V"]).astype(np.int64)
    E = np.asarray(inputs["E"]).astype(np.int64)
    X = np.asarray(inputs["X"], np.float32)
    S = np.asarray(inputs["S"], np.float32)
    bf = np.dtype("bfloat16")

    deg_v = np.bincount(V, minlength=N_N).astype(np.float64)
    cnt_e = np.bincount(E, minlength=N_M).astype(np.float64)
    deginv = np.where(deg_v > 0, 1.0 / np.maximum(deg_v, 1.0), 0.0)
    De = np.zeros(N_M, np.float64)
    np.add.at(De, E, deg_v[V])
    De = De / (cnt_e + 1.0)
    De_inv = np.where(De > 0, De ** -0.5, 1.0)
    coef_e = np.where(cnt_e > 0, De_inv / np.maximum(cnt_e, 1.0), 0.0)
    with np.errstate(divide="ignore"):
        Dv_inv = np.where(deg_v > 0, deg_v ** -0.5, 0.0)

    tg_all = _tgid_node(V)
    ge_all = _gid_edge(E)
    owner_e = E // ES_OWN
    r_e = E - owner_e * ES_OWN
    owner_v = V // NS
    u_v = V - owner_v * NS

    v2e_a_raw, v2e_b_raw, e2v_raw = [], [], []
    for c in range(NCORES):
        m = owner_e == c
        dst = r_e[m]
        src = tg_all[m]
        mA = src < SPLIT
        v2e_a_raw.append((dst[mA], src[mA]))
        v2e_b_raw.append((dst[~mA], src[~mA] - SPLIT))
        mv = owner_v == c
        e2v_raw.append((u_v[mv], ge_all[mv]))

    cpt_va = _caps(v2e_a_raw, NT_EO, min1=True)
    cpt_vb = _caps(v2e_b_raw, NT_EO, min1=False)
    cpt_e = _caps(e2v_raw, NT_V, min1=True)
    LvA = int(np.sum(cpt_va) * P)
    LvB = int(np.sum(cpt_vb) * P)
    Le = int(np.sum(cpt_e) * P)
    LvAP = ((LvA + NI - 1) // NI) * NI
    LvBP = max(NI, ((LvB + NI - 1) // NI) * NI)
    LeP = ((Le + NI - 1) // NI) * NI

    g = lambda k: np.asarray(inputs[k], np.float32)
    W = {}
    # layer 0
    Wv0, bv0, a0 = g("Wv0"), g("bv0"), g("a0")
    W["va0"] = np.ascontiguousarray((Wv0 @ a0)[:, None]).astype(bf)
    c0 = float(bv0 @ a0)
    W["Wv0t"] = np.ascontiguousarray(Wv0).astype(bf)
    W["bv0t"] = np.tile(bv0[None, :], (P, 1)).astype(np.float32)
    Wx0, bx0 = g("Wx0"), g("bx0")
    W["Wx0h0"] = np.ascontiguousarray(Wx0).astype(bf)
    W["bx0m1"] = np.tile((bx0 - 1.0)[None, :], (P, 1))
    # layer 1
    Wv1, bv1, a1 = g("Wv1"), g("bv1"), g("a1")
    va1 = Wv1 @ a1
    W["va1h0"] = np.ascontiguousarray(va1[:128, None]).astype(bf)
    W["va1h1"] = np.ascontiguousarray(va1[128:, None]).astype(bf)
    c1 = float(bv1 @ a1)
    W["Wv1h0"] = np.ascontiguousarray(Wv1[:128]).astype(bf)
    W["Wv1h1"] = np.ascontiguousarray(Wv1[128:]).astype(bf)
    W["bv1t"] = np.tile(bv1[None, :], (P, 1)).astype(np.float32)
    Wx1, bx1 = g("Wx1"), g("bx1")
    W["Wx1h0"] = np.ascontiguousarray(Wx1[:128]).astype(bf)
    W["Wx1h1"] = np.ascontiguousarray(Wx1[128:]).astype(bf)
    W["bx1m1"] = np.tile((bx1 - 1.0)[None, :], (P, 1))
    for l in range(2):
        Wt, bt = g(f"Wt{l}"), g(f"bt{l}")
        btf = bt - Wt[:256].sum(axis=0)
        W[f"Wt{l}h0"] = np.ascontiguousarray(Wt[:128]).astype(bf)
        W[f"Wt{l}h1"] = np.ascontiguousarray(Wt[128:256]).astype(bf)
        W[f"Wt{l}bot"] = np.ascontiguousarray(Wt[256:]).astype(bf)
        W[f"bt{l}f"] = np.tile(btf[None, :].astype(np.float32), (P, 1))
    Wf = g("Wf")
    W["Wfh0"] = np.ascontiguousarray(Wf[:128]).astype(bf)
    W["Wfh1"] = np.ascontiguousarray(Wf[128:]).astype(bf)
    W["bft"] = np.tile(g("bf")[None, :], (P, 1))

    iota = np.tile(np.arange(P, dtype=np.float32)[None, :], (P, 1))
    iota_rep = np.ascontiguousarray(
        np.broadcast_to(iota[:, None, :], (P, WCH, P))).astype(bf)

    # full X, block-major padded layout (col c*6272+u)
    XTg = np.zeros((128, NGP), np.float32)
    for c in range(NCORES):
        XTg[:, c * NSP:c * NSP + NS] = X[c * NS:(c + 1) * NS].T
    XTg = XTg.astype(bf)

    in_maps = []
    for c in range(NCORES):
        gva, ecva = _build_stream(*v2e_a_raw[c], NT_EO, cpt_va)
        gvb, ecvb = _build_stream(*v2e_b_raw[c], NT_EO, cpt_vb)
        ge_, ece = _build_stream(*e2v_raw[c], NT_V, cpt_e)
        gva_p, ecva_2d = _pad_stream(gva, ecva, LvAP)
        gvb_p, ecvb_2d = _pad_stream(gvb, ecvb, LvBP)
        ge_p, ece_2d = _pad_stream(ge_, ece, LeP)

        e0 = c * ES_OWN
        ST_own = np.zeros((MS_OWN, 64), np.float32)
        ST_own[:ES_OWN] = S[e0:e0 + ES_OWN]
        coef_own = np.zeros(MS_OWN, np.float32)
        coef_own[:ES_OWN] = coef_e[e0:e0 + ES_OWN]

        im = dict(
            XT=XTg,
            XTl=np.ascontiguousarray(XTg[:, c * NSP:(c + 1) * NSP]),
            ST=np.ascontiguousarray(ST_own.T).astype(bf),
            gva_idx=_wrap_idx(gva_p), gvb_idx=_wrap_idx(gvb_p),
            ge_idx=_wrap_idx(ge_p),
            ec_va=ecva_2d, ec_vb=ecvb_2d, ec_e=ece_2d,
            iota_rep=iota_rep,
            cvec=np.tile(np.array([[c0, c1, -1.0, 0.0]], np.float32),
                         (P, 1)),
            dgi=_cols(deginv[c * NS:(c + 1) * NS], NT_V),
            dvi=_cols(Dv_inv[c * NS:(c + 1) * NS], NT_V),
            cf=_cols(coef_own, NT_EO),
        )
        im.update(W)
        in_maps.append(im)

    meta = dict(cpt_va=[int(x) for x in cpt_va],
                cpt_vb=[int(x) for x in cpt_vb],
                cpt_e=[int(x) for x in cpt_e],
                LvAP=LvAP, LvBP=LvBP, LeP=LeP,
                regs_va=_regs(LvA, LvAP), regs_vb=_regs(LvB, LvBP),
                regs_e=_regs(Le, LeP), c0=c0, c1=c1)
    return in_maps, meta


# ---------------------------------------------------------------------------

def build_program(meta):
    nc = bacc.Bacc("TRN2", target_bir_lowering=False, debug=False,
                   num_devices=NCORES, num_swdge_queues=GQ)

    def din(name, shape, dt=F32):
        return nc.dram_tensor(name, shape, dt, kind="ExternalInput")

    XT = din("XT", [P, NGP], BF16)
    XTl = din("XTl", [P, NSP], BF16)
    ST = din("ST", [64, MS_OWN], BF16)
    gva_idx = din("gva_idx", [P, meta["LvAP"] // 16], I16)
    gvb_idx = din("gvb_idx", [P, meta["LvBP"] // 16], I16)
    ge_idx = din("ge_idx", [P, meta["LeP"] // 16], I16)
    nch_va = meta["LvAP"] // P
    nch_vb = meta["LvBP"] // P
    nch_e = meta["LeP"] // P
    ec_va = din("ec_va", [P, nch_va], BF16)
    ec_vb = din("ec_vb", [P, nch_vb], BF16)
    ec_e = din("ec_e", [P, nch_e], BF16)
    iota_rep = din("iota_rep", [P, WCH, P], BF16)
    cvec = din("cvec", [P, 4])
    dgi = din("dgi", [P, NT_V])
    dvi = din("dvi", [P, NT_V])
    cf = din("cf", [P, NT_EO])
    wshapes = dict(va0=[P, 1], Wv0t=[P, 256], Wx0h0=[P, 256],
                   va1h0=[P, 1], va1h1=[P, 1],
                   Wv1h0=[P, 256], Wv1h1=[P, 256],
                   Wx1h0=[P, 256], Wx1h1=[P, 256],
                   Wt0h0=[P, 256], Wt0h1=[P, 256], Wt0bot=[64, 256],
                   Wt1h0=[P, 256], Wt1h1=[P, 256], Wt1bot=[64, 256],
                   Wfh0=[P, 128], Wfh1=[P, 128])
    fshapes = dict(bv0t=[P, 256], bx0m1=[P, 256], bv1t=[P, 256],
                   bx1m1=[P, 256], bt0f=[P, 256], bt1f=[P, 256],
                   bft=[P, 128])
    Wd = {k: din(k, s, BF16) for k, s in wshapes.items()}
    Wd.update({k: din(k, s, F32) for k, s in fshapes.items()})

    yout = nc.dram_tensor("yout", [NS, 128], F32, kind="ExternalOutput")

    rg = [list(range(NCORES))]
    ag_space = "Shared" if SHARED_AG else "Local"

    with tile.TileContext(nc) as tc:
        ctx = ExitStack()
        sbuf = ctx.enter_context(tc.tile_pool(name="sbuf", bufs=2))
        psum = ctx.enter_context(tc.tile_pool(name="psum", bufs=2, space="PSUM"))
        dram = ctx.enter_context(tc.tile_pool(name="dram", bufs=1, space="DRAM"))
        cons = ctx.enter_context(tc.tile_pool(name="cons", bufs=1))

        iota_t = cons.tile([P, WCH, P], BF16, name="iota_t")
        nc.scalar.dma_start(iota_t[:], iota_rep[:])
        ident = cons.tile([P, P], F32, name="ident")
        make_identity(nc, ident[:])
        wt = {}
        for k, h in Wd.items():
            t_ = cons.tile(list(h.shape), h.dtype, name=f"w_{k}")
            nc.scalar.dma_start(t_[:], h[:])
            wt[k] = t_
        st_t = cons.tile([64, MS_OWN], BF16, name="st_t")
        nc.sync.dma_start(st_t[:], ST[:])
        ecva_t = cons.tile([P, nch_va], BF16, name="ecva_t")
        nc.scalar.dma_start(ecva_t[:], ec_va[:])
        ecvb_t = cons.tile([P, nch_vb], BF16, name="ecvb_t")
        nc.scalar.dma_start(ecvb_t[:], ec_vb[:])
        ece_t = cons.tile([P, nch_e], BF16, name="ece_t")
        nc.scalar.dma_start(ece_t[:], ec_e[:])
        gva_t = cons.tile([P, meta["LvAP"] // 16], I16, name="gva_t")
        nc.sync.dma_start(gva_t[:], gva_idx[:])
        gvb_t = cons.tile([P, meta["LvBP"] // 16], I16, name="gvb_t")
        nc.sync.dma_start(gvb_t[:], gvb_idx[:])
        ge_t = cons.tile([P, meta["LeP"] // 16], I16, name="ge_t")
        nc.sync.dma_start(ge_t[:], ge_idx[:])
        cvec_t = cons.tile([P, 4], F32, name="cvec_t")
        nc.scalar.dma_start(cvec_t[:], cvec[:])
        dgi_t = cons.tile([P, NT_V], F32, name="dgi_t")
        nc.scalar.dma_start(dgi_t[:], dgi[:])
        dgin_t = cons.tile([P, NT_V], F32, name="dgin_t")
        nc.vector.tensor_scalar_mul(out=dgin_t[:], in0=dgi_t[:], scalar1=-1.0)
        dvi_t = cons.tile([P, NT_V], F32, name="dvi_t")
        nc.scalar.dma_start(dvi_t[:], dvi[:])
        cf_t = cons.tile([P, NT_EO], F32, name="cf_t")
        nc.scalar.dma_start(cf_t[:], cf[:])

        # node-side x_init state lives in DRAM (SBUF is tight)
        sc_sb = cons.tile([P, NT_V], F32, name="sc_sb")
        xinit0_d = dram.tile([NSP, 256], BF16, name="xinit0_d")
        xinit1_d = dram.tile([NSP, 256], BF16, name="xinit1_d")

        # DRAM tables
        tab0 = dram.tile([NGP, 256], BF16, name="tab0")
        tab1l = dram.tile([NSP, 384], BF16, name="tab1l")
        tab1gA = dram.tile([SPLIT, 384], BF16, name="tab1gA",
                           addr_space=ag_space)
        tab1gB = dram.tile([NGB, 384], BF16, name="tab1gB",
                           addr_space=ag_space)
        tab2l = dram.tile([NSP, 128], BF16, name="tab2l")
        tab2gA = dram.tile([SPLIT, 128], BF16, name="tab2gA",
                           addr_space=ag_space)
        tab2gB = dram.tile([NGB, 128], BF16, name="tab2gB",
                           addr_space=ag_space)
        ytl = [dram.tile([MS_OWN, 256], BF16, name=f"ytl{l}") for l in range(2)]
        ytg = [dram.tile([ME, 256], BF16, name=f"ytg{l}", addr_space=ag_space)
               for l in range(2)]
        yt3l = dram.tile([MS_OWN, 128], BF16, name="yt3l")
        yt3g = dram.tile([ME, 128], BF16, name="yt3g", addr_space=ag_space)

        qctr = [0]

        def cc_ag(in_ap, out_ap):
            bass.BassGpSimd.collective_compute(
                nc.gpsimd, "AllGather", mybir.AluOpType.bypass,
                replica_groups=rg, ins=[in_ap], outs=[out_ap])

        def scatter_pass(streams, used_cols, n_tiles, on_tile):
            """Gather + one-hot-matmul segment sum over dest tiles."""
            S_ = len(streams)
            chunk_lists = []
            for st in streams:
                tof = []
                for t, n in enumerate(st["cpt"]):
                    tof += [t] * n
                chunk_lists.append(tof)
            order = []
            ks = [0] * S_
            for t in range(n_tiles):
                for s in range(S_):
                    for _ in range(streams[s]["cpt"][t]):
                        order.append((s, ks[s]))
                        ks[s] += 1
            first_c, last_c = {}, {}
            for pos, (s, k) in enumerate(order):
                t = chunk_lists[s][k]
                first_c.setdefault(t, pos)
                last_c[t] = pos
            g_tiles = [[None] * len(st["regs"]) for st in streams]
            emitted = [0] * S_

            def ensure_emitted(s, upto):
                st = streams[s]
                while emitted[s] <= min(upto, len(st["regs"]) - 1):
                    call = emitted[s]
                    if st["regs"][call] > 0:
                        gt = sbuf.tile([P, WCH, st["es"]], BF16,
                                       tag=st["ring"], bufs=st["gb"],
                                       name=f"g{st['tag']}_{call}")
                        nc.gpsimd.dma_gather(
                            out_ap=gt[:], in_ap=st["in_ap"],
                            idxs_ap=st["idx_t"][:, call * (NI // 16):
                                                (call + 1) * (NI // 16)],
                            num_idxs=NI, num_idxs_reg=st["regs"][call],
                            elem_size=st["es"], queue_num=qctr[0] % GQ)
                        qctr[0] += 1
                        g_tiles[s][call] = gt
                    emitted[s] += 1

            a_cur = [[None, -1] for _ in range(S_)]
            mega = [None, -1]
            for pos, (s, k) in enumerate(order):
                st = streams[s]
                t = chunk_lists[s][k]
                call, j = k // WCH, k % WCH
                ensure_emitted(s, call + st["gb"] - 1)
                gt = g_tiles[s][call]
                if gt is None:
                    continue
                w = k // WCH
                if a_cur[s][1] != w:
                    ab = sbuf.tile([P, WCH, P], BF16, tag=f"A{s}", bufs=2,
                                   name=f"A{st['tag']}_{w}")
                    nc.vector.tensor_tensor(
                        out=ab[:],
                        in0=st["ec_t"][:, w * WCH:(w + 1) * WCH].to_broadcast(
                            [P, WCH, P]),
                        in1=iota_t[:],
                        op=mybir.AluOpType.is_equal)
                    a_cur[s] = [ab, w]
                mw = t // PSW
                if mega[1] != mw:
                    mega = [psum.tile([P, PSW, 512], F32, tag="ps", bufs=2,
                                      name=f"ps{st['tag']}_{mw}"), mw]
                pt = mega[0]
                q = t % PSW
                nc.tensor.matmul(
                    out=pt[:, q, 0:used_cols],
                    lhsT=a_cur[s][0][:, j, :],
                    rhs=gt[:, j, 0:used_cols],
                    start=(pos == first_c[t]), stop=(pos == last_c[t]))
                if pos == last_c[t]:
                    on_tile(t, pt, q)

        def elu_u(z_ap, w, cols, tag, i):
            """relu(z) + exp(min(z,0)) = elu(z) + 1 (2 DVE + 2 ACT ops)."""
            mn = sbuf.tile([P, cols], F32, tag="mn", bufs=2, name=f"mn{tag}{i}")
            nc.vector.tensor_scalar_min(out=mn[:w], in0=z_ap, scalar1=0.0)
            ex = sbuf.tile([P, cols], F32, tag="ex", bufs=2, name=f"ex{tag}{i}")
            nc.scalar.activation(ex[:w], mn[:w], AF.Exp)
            rl = sbuf.tile([P, cols], F32, tag="rl", bufs=2, name=f"rl{tag}{i}")
            nc.scalar.activation(rl[:w], z_ap, AF.Relu)
            u = sbuf.tile([P, cols], F32, tag="u", bufs=2, name=f"u{tag}{i}")
            nc.vector.tensor_add(u[:w], rl[:w], ex[:w])
            return u

        def elu_u_psum(pt_ap, scale_ap, nscale_ap, tag, i):
            """elu(psum*scale) + 1 from PSUM: 3 ACT + 1 DVE, no z staging."""
            rl = sbuf.tile([P, 256], F32, tag="rl", bufs=2, name=f"rl{tag}{i}")
            nc.scalar.activation(rl[:], pt_ap, AF.Relu, scale=scale_ap)
            r2 = sbuf.tile([P, 256], F32, tag="r2", bufs=2, name=f"r2{tag}{i}")
            nc.scalar.activation(r2[:], pt_ap, AF.Relu, scale=nscale_ap)
            ex = sbuf.tile([P, 256], F32, tag="ex", bufs=2, name=f"ex{tag}{i}")
            nc.scalar.activation(ex[:], r2[:], AF.Exp,
                                 scale=cvec_t[:, 2:3])
            u = sbuf.tile([P, 256], F32, tag="u", bufs=2, name=f"u{tag}{i}")
            nc.vector.tensor_add(u[:], rl[:], ex[:])
            return u

        def transpose_pair(src_ap, tag, i):
            """[P,256] f32 -> two [P,P] bf16 transposed tiles (copies on ACT)."""
            pT = psum.tile([P, 512], F32, tag="pT", bufs=2,
                           name=f"pT{tag}_{i}")
            outs = []
            for hi in range(2):
                nc.tensor.transpose(out=pT[:, hi * P:(hi + 1) * P],
                                    in_=src_ap[:, hi * P:(hi + 1) * P],
                                    identity=ident[:, :])
                sT = sbuf.tile([P, P], BF16, tag="sT", bufs=4,
                               name=f"sT{tag}_{i}_{hi}")
                nc.scalar.activation(sT[:], pT[:, hi * P:(hi + 1) * P],
                                     AF.Copy)
                outs.append(sT)
            return outs

        def wt_matmuls(l, t, uT, tag):
            py = psum.tile([P, 512], F32, tag="pd", bufs=2, name=f"py{tag}_{t}")
            nc.tensor.matmul(out=py[:, 0:256],
                             lhsT=st_t[:, t * P:(t + 1) * P],
                             rhs=wt[f"Wt{l}bot"][:], start=True, stop=False)
            nc.tensor.matmul(out=py[:, 0:256], lhsT=uT[0][:],
                             rhs=wt[f"Wt{l}h0"][:], start=False, stop=False)
            nc.tensor.matmul(out=py[:, 0:256], lhsT=uT[1][:],
                             rhs=wt[f"Wt{l}h1"][:], start=False, stop=True)
            return py

        # ------------------------------------------------------------------
        # stage 1: build full layer-0 table [ew*X | ew] + local xinit0
        ident_bf = cons.tile([P, P], BF16, name="ident_bf")
        nc.vector.tensor_copy(out=ident_bf[:], in_=ident[:])
        # local xinit0 first: its table writes overlap stage-1 compute
        # instead of colliding with stage-2's gather stream
        for kk in range(K_T):
            xtg = sbuf.tile([P, TCH_L], BF16, tag="xtg", bufs=3,
                            name=f"xtlg_{kk}")
            nc.sync.dma_start(xtg[:], XTl[:, kk * TCH_L:(kk + 1) * TCH_L])
            for j in range(K_T):
                t = kk * K_T + j
                pi = psum.tile([P, 512], F32, tag="pd", bufs=2,
                               name=f"pi0_{t}")
                nc.tensor.matmul(out=pi[:, 0:256],
                                 lhsT=xtg[:, j * P:(j + 1) * P],
                                 rhs=wt["Wx0h0"][:], start=True, stop=True)
                xi0 = sbuf.tile([P, 256], BF16, tag="xi", bufs=2,
                                name=f"xi0_{t}")
                nc.vector.tensor_add(xi0[:], pi[:, 0:256], wt["bx0m1"][:])
                nc.sync.dma_start(out=xinit0_d[t * P:(t + 1) * P, :],
                                  in_=xi0[:])
        # A-half table groups (kk 0-3) of every core first, so stream-A
        # gathers can start before the B-half rows are written
        for c8, kk in ([(c, k) for k in range(4) for c in range(NCORES)]
                       + [(c, k) for k in range(4, K_T)
                          for c in range(NCORES)]):
            if True:
                ps7 = psum.tile([P, 512], F32, tag="pd", bufs=2,
                                name=f"ps7_{c8}_{kk}")
                col0 = c8 * NSP + kk * TCH_L
                xtg = sbuf.tile([P, TCH_L], BF16, tag="xtg", bufs=3,
                                name=f"xtg_{c8}_{kk}")
                nc.sync.dma_start(xtg[:], XT[:, col0:col0 + TCH_L])
                for j in range(K_T):
                    nc.tensor.matmul(out=ps7[:, j:j + 1],
                                     lhsT=xtg[:, j * P:(j + 1) * P],
                                     rhs=wt["va0"][:], start=True, stop=True)
                sc7 = sbuf.tile([P, K_T], F32, tag="sc7", bufs=2,
                                name=f"sc7_{c8}_{kk}")
                nc.vector.tensor_scalar_add(out=sc7[:], in0=ps7[:, 0:K_T],
                                            scalar1=meta["c0"])
                lr7 = sbuf.tile([P, K_T], F32, tag="lr7", bufs=2,
                                name=f"lr7_{c8}_{kk}")
                nc.vector.tensor_scalar_mul(out=lr7[:], in0=sc7[:],
                                            scalar1=NEG_SLOPE)
                mx7 = sbuf.tile([P, K_T], F32, tag="mx7", bufs=2,
                                name=f"mx7_{c8}_{kk}")
                nc.vector.tensor_tensor(out=mx7[:], in0=sc7[:], in1=lr7[:],
                                        op=mybir.AluOpType.max)
                ew7 = sbuf.tile([P, K_T], F32, tag="ew7", bufs=2,
                                name=f"ew7_{c8}_{kk}")
                nc.scalar.activation(ew7[:], mx7[:], AF.Exp)
                stg = sbuf.tile([P, K_T, 256], BF16, tag="stg0", bufs=3,
                                name=f"stg0_{c8}_{kk}")
                for j0, nj in ((0, 4), (4, 3)):
                    pTq = psum.tile([P, 512], BF16, tag="pT", bufs=2,
                                    name=f"xpT_{c8}_{kk}_{j0}")
                    for j in range(nj):
                        nc.tensor.transpose(
                            out=pTq[:, j * P:(j + 1) * P],
                            in_=xtg[:, (j0 + j) * P:(j0 + j + 1) * P],
                            identity=ident_bf[:, :])
                    nc.vector.tensor_tensor(
                        out=stg[:, j0:j0 + nj, 0:128],
                        in0=pTq[:, 0:nj * P].rearrange("p (j c) -> p j c", j=nj),
                        in1=ew7[:, j0:j0 + nj].to_broadcast([P, nj, P]),
                        op=mybir.AluOpType.mult)
                nc.vector.tensor_copy(out=stg[:, :, 128:129],
                                      in_=ew7[:].to_broadcast([P, K_T, 1]))
                if kk < 4:
                    r0 = c8 * TA_L + kk * TCH_L
                else:
                    r0 = SPLIT + c8 * TB_L + (kk - 4) * TCH_L
                nc.sync.dma_start(
                    out=tab0[r0:r0 + TCH_L, :].rearrange(
                        "(j p) c -> p j c", p=P),
                    in_=stg[:])

        # ------------------------------------------------------------------
        # stage 2: L0 v2e scatter into owned edges + fused edge epilogue
        def v2e0_tile(t, pt, q):
            dc = sbuf.tile([P, 1], F32, tag="dc", bufs=2, name=f"dc0_{t}")
            nc.vector.tensor_scalar_max(out=dc[:], in0=pt[:, q, 128:129],
                                        scalar1=1e-35)
            di = sbuf.tile([P, 1], F32, tag="di", bufs=2, name=f"di0_{t}")
            nc.vector.reciprocal(di[:], dc[:])
            zx = sbuf.tile([P, P], F32, tag="zx", bufs=2, name=f"zx0_{t}")
            nc.scalar.activation(zx[:], pt[:, q, 0:128], AF.Copy,
                                 scale=di[:, :])
            pT = psum.tile([P, 512], F32, tag="pT", bufs=2, name=f"zxT_{t}")
            nc.tensor.transpose(out=pT[:, 0:P], in_=zx[:], identity=ident[:, :])
            zxT = sbuf.tile([P, P], BF16, tag="sT", bufs=4, name=f"zxTs_{t}")
            nc.scalar.activation(zxT[:], pT[:, 0:P], AF.Copy)
            pz = psum.tile([P, 512], F32, tag="pd", bufs=2, name=f"pz0_{t}")
            nc.tensor.matmul(out=pz[:, 0:256], lhsT=zxT[:], rhs=wt["Wv0t"][:],
                             start=True, stop=True)
            F = sbuf.tile([P, 256], F32, tag="F", bufs=2, name=f"F0_{t}")
            nc.vector.tensor_add(F[:], pz[:, 0:256], wt["bv0t"][:])
            u = elu_u(F[:], P, 256, "e0", t)
            uT = transpose_pair(u[:], "u0", t)
            py = wt_matmuls(0, t, uT, "y0")
            yt = sbuf.tile([P, 256], BF16, tag="yt", bufs=2, name=f"yt0_{t}")
            nc.vector.tensor_add(yt[:], py[:, 0:256], wt["bt0f"][:])
            nc.sync.dma_start(out=ytl[0][t * P:(t + 1) * P, :], in_=yt[:])
            if t % 5 == 4:
                k = t // 5
                cc_ag(ytl[0][k * OWNR:(k + 1) * OWNR, :],
                      ytg[0][k * CH_E:(k + 1) * CH_E, :])

        scatter_pass(
            [dict(in_ap=tab0[0:SPLIT, :], es=256, idx_t=gva_t, ec_t=ecva_t,
                  cpt=meta["cpt_va"], regs=meta["regs_va"], tag="va0",
                  gb=GB_A, ring="gva"),
             dict(in_ap=tab0[SPLIT:NGP, :], es=256, idx_t=gvb_t, ec_t=ecvb_t,
                  cpt=meta["cpt_vb"], regs=meta["regs_vb"], tag="vb0",
                  gb=GB_B, ring="gvb")],
            129, NT_EO, v2e0_tile)

        # ------------------------------------------------------------------
        # stage 3: L0 e2v + h1/hT/xinit1 + local L1 table + AG
        h_tiles = {}

        def flush_tab1(t):
            """Premultiply h tiles of the finished 7-tile group, write+AG."""
            nj = t % K_T + 1
            kk = t // K_T
            t0 = t - nj + 1
            sc7 = sbuf.tile([P, K_T], F32, tag="sc7", bufs=2, name=f"sc1_{kk}")
            nc.vector.tensor_scalar_add(out=sc7[:, 0:nj],
                                        in0=sc_sb[:, t0:t + 1],
                                        scalar1=meta["c1"])
            lr7 = sbuf.tile([P, K_T], F32, tag="lr7", bufs=2, name=f"lr1_{kk}")
            nc.vector.tensor_scalar_mul(out=lr7[:, 0:nj], in0=sc7[:, 0:nj],
                                        scalar1=NEG_SLOPE)
            mx7 = sbuf.tile([P, K_T], F32, tag="mx7", bufs=2, name=f"mx1_{kk}")
            nc.vector.tensor_tensor(out=mx7[:, 0:nj], in0=sc7[:, 0:nj],
                                    in1=lr7[:, 0:nj], op=mybir.AluOpType.max)
            ew7 = sbuf.tile([P, K_T], F32, tag="ew7", bufs=2, name=f"ew1_{kk}")
            nc.scalar.activation(ew7[:, 0:nj], mx7[:, 0:nj], AF.Exp)
            stg = sbuf.tile([P, K_T, 384], BF16, tag="stg1", bufs=2,
                            name=f"stg1_{kk}")
            for jj in range(nj):
                tt = t0 + jj
                h = h_tiles.pop(tt)
                nc.vector.tensor_scalar_mul(out=stg[:, jj, 0:256], in0=h[:],
                                            scalar1=ew7[:, jj:jj + 1])
            nc.vector.tensor_copy(
                out=stg[:, 0:nj, 256:257],
                in_=ew7[:, 0:nj].to_broadcast([P, nj, 1]))
            r0 = kk * TCH_L
            nc.sync.dma_start(
                out=tab1l[r0:r0 + nj * P, :].rearrange("(j p) c -> p j c", p=P),
                in_=stg[:, 0:nj, :])
            if kk == 3:
                cc_ag(tab1l[0:TA_L, :], tab1gA[:])
            elif kk == K_T - 1:
                cc_ag(tab1l[TA_L:NSP, :], tab1gB[:])

        def e2v0_tile(t, pt, q):
            xi0 = sbuf.tile([P, 256], BF16, tag="xil", bufs=3, name=f"xi0l_{t}")
            nc.scalar.dma_start(xi0[:], xinit0_d[t * P:(t + 1) * P, :])
            u = elu_u_psum(pt[:, q, 0:256], dgi_t[:, t:t + 1],
                           dgin_t[:, t:t + 1], "n0", t)
            h = sbuf.tile([P, 256], F32, tag="h", bufs=8, name=f"h1_{t}")
            nc.vector.tensor_add(h[:], u[:], xi0[:])
            h_tiles[t] = h
            hTt = transpose_pair(h[:], "h1", t)
            pi = psum.tile([P, 512], F32, tag="pd", bufs=2, name=f"pi1_{t}")
            nc.tensor.matmul(out=pi[:, 0:256], lhsT=hTt[0][:],
                             rhs=wt["Wx1h0"][:], start=True, stop=False)
            nc.tensor.matmul(out=pi[:, 0:256], lhsT=hTt[1][:],
                             rhs=wt["Wx1h1"][:], start=False, stop=True)
            xi1 = sbuf.tile([P, 256], BF16, tag="xi", bufs=2, name=f"xi1_{t}")
            nc.vector.tensor_add(xi1[:], pi[:, 0:256], wt["bx1m1"][:])
            nc.sync.dma_start(out=xinit1_d[t * P:(t + 1) * P, :], in_=xi1[:])
            nc.tensor.matmul(out=pi[:, 256:257], lhsT=hTt[0][:],
                             rhs=wt["va1h0"][:], start=True, stop=False)
            nc.tensor.matmul(out=pi[:, 256:257], lhsT=hTt[1][:],
                             rhs=wt["va1h1"][:], start=False, stop=True)
            nc.vector.tensor_copy(out=sc_sb[:, t:t + 1], in_=pi[:, 256:257])
            if t % K_T == K_T - 1:
                flush_tab1(t)

        scatter_pass(
            [dict(in_ap=ytg[0][:], es=256, idx_t=ge_t, ec_t=ece_t,
                  cpt=meta["cpt_e"], regs=meta["regs_e"], tag="e0",
                  gb=GB_E, ring="ge")],
            256, NT_V, e2v0_tile)

        # ------------------------------------------------------------------
        # stage 4: L1 v2e + fused edge epilogue
        def v2e1_tile(t, pt, q):
            dc = sbuf.tile([P, 1], F32, tag="dc", bufs=2, name=f"dc1_{t}")
            nc.vector.tensor_scalar_max(out=dc[:], in0=pt[:, q, 256:257],
                                        scalar1=1e-35)
            di = sbuf.tile([P, 1], F32, tag="di", bufs=2, name=f"di1_{t}")
            nc.vector.reciprocal(di[:], dc[:])
            zd = sbuf.tile([P, 256], F32, tag="zd", bufs=2, name=f"zd1_{t}")
            nc.scalar.activation(zd[:], pt[:, q, 0:256], AF.Copy,
                                 scale=di[:, :])
            zT = transpose_pair(zd[:], "z1", t)
            pz = psum.tile([P, 512], F32, tag="pd", bufs=2, name=f"pz1_{t}")
            nc.tensor.matmul(out=pz[:, 0:256], lhsT=zT[0][:],
                             rhs=wt["Wv1h0"][:], start=True, stop=False)
            nc.tensor.matmul(out=pz[:, 0:256], lhsT=zT[1][:],
                             rhs=wt["Wv1h1"][:], start=False, stop=True)
            F = sbuf.tile([P, 256], F32, tag="F", bufs=2, name=f"F1_{t}")
            nc.vector.tensor_add(F[:], pz[:, 0:256], wt["bv1t"][:])
            u = elu_u(F[:], P, 256, "e1", t)
            uT = transpose_pair(u[:], "u1", t)
            py = wt_matmuls(1, t, uT, "y1")
            yt = sbuf.tile([P, 256], BF16, tag="yt", bufs=2, name=f"yt1_{t}")
            nc.vector.tensor_add(yt[:], py[:, 0:256], wt["bt1f"][:])
            nc.sync.dma_start(out=ytl[1][t * P:(t + 1) * P, :], in_=yt[:])
            if t % 5 == 4:
                k = t // 5
                cc_ag(ytl[1][k * OWNR:(k + 1) * OWNR, :],
                      ytg[1][k * CH_E:(k + 1) * CH_E, :])

        scatter_pass(
            [dict(in_ap=tab1g[0:SPLIT, :], es=384, idx_t=gva_t, ec_t=ecva_t,
                  cpt=meta["cpt_va"], regs=meta["regs_va"], tag="va1",
                  gb=GB_A, ring="gva1"),
             dict(in_ap=tab1g[SPLIT:NGP, :], es=384, idx_t=gvb_t, ec_t=ecvb_t,
                  cpt=meta["cpt_vb"], regs=meta["regs_vb"], tag="vb1",
                  gb=GB_B, ring="gvb1")],
            257, NT_EO, v2e1_tile)

        # ------------------------------------------------------------------
        # stage 5: L1 e2v -> h2 -> local T2 table + AG
        stg2 = [None]

        def e2v1_tile(t, pt, q):
            xi1 = sbuf.tile([P, 256], BF16, tag="xil", bufs=3, name=f"xi1l_{t}")
            nc.scalar.dma_start(xi1[:], xinit1_d[t * P:(t + 1) * P, :])
            u = elu_u_psum(pt[:, q, 0:256], dgi_t[:, t:t + 1],
                           dgin_t[:, t:t + 1], "n1", t)
            h2 = sbuf.tile([P, 256], F32, tag="h", bufs=8, name=f"h2_{t}")
            nc.vector.tensor_add(h2[:], u[:], xi1[:])
            h2T = transpose_pair(h2[:], "h2", t)
            pf = psum.tile([P, 512], F32, tag="pd", bufs=2, name=f"pf2_{t}")
            nc.tensor.matmul(out=pf[:, 0:128], lhsT=h2T[0][:],
                             rhs=wt["Wfh0"][:], start=True, stop=False)
            nc.tensor.matmul(out=pf[:, 0:128], lhsT=h2T[1][:],
                             rhs=wt["Wfh1"][:], start=False, stop=True)
            if stg2[0] is None:
                stg2[0] = sbuf.tile([P, K_T, 128], BF16, tag="stg2", bufs=2,
                                    name=f"stg2_{t}")
            jj = t % K_T
            nc.vector.tensor_add(stg2[0][:, jj, :], pf[:, 0:128], wt["bft"][:])
            if jj == K_T - 1:
                kk = t // K_T
                r0 = kk * TCH_L
                nc.sync.dma_start(
                    out=tab2l[r0:r0 + TCH_L, :].rearrange(
                        "(j p) c -> p j c", p=P),
                    in_=stg2[0][:])
                stg2[0] = None
                if kk == 3:
                    cc_ag(tab2l[0:TA_L, :], tab2gA[:])
                elif kk == K_T - 1:
                    cc_ag(tab2l[TA_L:NSP, :], tab2gB[:])

        scatter_pass(
            [dict(in_ap=ytg[1][:], es=256, idx_t=ge_t, ec_t=ece_t,
                  cpt=meta["cpt_e"], regs=meta["regs_e"], tag="e1",
                  gb=GB_E, ring="ge")],
            256, NT_V, e2v1_tile)

        # ------------------------------------------------------------------
        # stage 6: HC v2e (linear: sum T2 rows, scale by coef)
        def v2e2_tile(t, pt, q):
            yt = sbuf.tile([P, 128], BF16, tag="yt3", bufs=2, name=f"yt3_{t}")
            nc.vector.tensor_scalar_mul(out=yt[:], in0=pt[:, q, 0:128],
                                        scalar1=cf_t[:, t:t + 1])
            nc.sync.dma_start(out=yt3l[t * P:(t + 1) * P, :], in_=yt[:])
            if t % 5 == 4:
                k = t // 5
                cc_ag(yt3l[k * OWNR:(k + 1) * OWNR, :],
                      yt3g[k * CH_E:(k + 1) * CH_E, :])

        scatter_pass(
            [dict(in_ap=tab2g[0:SPLIT, :], es=128, idx_t=gva_t, ec_t=ecva_t,
                  cpt=meta["cpt_va"], regs=meta["regs_va"], tag="va2",
                  gb=GB_A, ring="gva2"),
             dict(in_ap=tab2g[SPLIT:NGP, :], es=128, idx_t=gvb_t, ec_t=ecvb_t,
                  cpt=meta["cpt_vb"], regs=meta["regs_vb"], tag="vb2",
                  gb=GB_B, ring="gvb2")],
            128, NT_EO, v2e2_tile)

        # ------------------------------------------------------------------
        # stage 7: HC e2v -> yout
        def e2v2_tile(t, pt, q):
            w = min(P, NS - t * P)
            ot = sbuf.tile([P, 128], F32, tag="fo", bufs=2, name=f"fo_{t}")
            nc.vector.tensor_scalar_mul(out=ot[:w], in0=pt[:w, q, 0:128],
                                        scalar1=dvi_t[:w, t:t + 1])
            nc.sync.dma_start(out=yout[t * P:t * P + w, :], in_=ot[:w])

        scatter_pass(
            [dict(in_ap=yt3g[:], es=128, idx_t=ge_t, ec_t=ece_t,
                  cpt=meta["cpt_e"], regs=meta["regs_e"], tag="e2",
                  gb=GB_E, ring="ge")],
            128, NT_V, e2v2_tile)
        ctx.close()

    nc.compile()
    return nc


_CACHED = {}


def kernel(**inputs):
    in_maps, meta = _prep(inputs)
    key = (meta["LvAP"], meta["LvBP"], meta["LeP"], tuple(meta["cpt_va"]),
           tuple(meta["cpt_vb"]), tuple(meta["cpt_e"]), meta["c0"], meta["c1"])
    if key not in _CACHED:
        _CACHED[key] = build_program(meta)
    nc = _CACHED[key]
    res = run_bass_kernel_spmd(nc, in_maps, list(range(NCORES)))
    out = np.concatenate([res.results[c]["yout"] for c in range(NCORES)],
                         axis=0)
    return np.ascontiguousarray(out.astype(np.float32))


# revision 37
# speedup vs baseline: 1.6566x; 1.0195x over previous
"""Trainium2 Bass kernel v3 for the 2-layer DPHGNN + hyperconv GNN stack.

Architecture (vs v2 baseline):
- v2e is EDGE-sharded: each core owns 2500 edges (20 tiles) and scatter-sums
  only into its owned edge rows -> no partial table over the full edge space
  and NO ReduceScatter anywhere.
- Node tables are AllGather'ed instead: each core builds the premultiplied
  table rows for its local nodes; chunked AllGathers (pipelined behind the
  e2v pass) replicate them.  Layer 0 needs no table collective at all: X is
  a kernel input, so every core builds the full table locally.
- Layer-0 table uses the low-rank trick: rows are [ew*X | ew] (129 cols,
  512B gather rows instead of 768B); Wv0 is applied post-aggregation.
- Edge epilogue is fused into the v2e scatter (PSUM -> Y tile directly);
  per-layer Y tables are AllGather'ed in 4 chunks as owned tiles complete.
- Hyperconv aggregates T2 = h2@Wf+bf rows (256B) and applies coef post-sum.
- Gather indices into the 50176-row node tables exceed int16, so v2e uses
  two streams (idx < 32768 and the rest, rebased).
"""

import sys
from contextlib import ExitStack

for _p in ("/opt/trn_rl_repo",):
    if _p not in sys.path:
        sys.path.append(_p)

import numpy as np

import concourse.bass as bass
import concourse.bacc as bacc
import concourse.mybir as mybir
import concourse.tile as tile
from concourse.bass_utils import run_bass_kernel_spmd
from concourse.masks import make_identity

F32 = mybir.dt.float32
BF16 = mybir.dt.bfloat16
I16 = mybir.dt.int16
AF = mybir.ActivationFunctionType

NEG_SLOPE = 0.2
P = 128
NCORES = 8
GQ = 4          # SWDGE queues
NI = 1024       # rows per dma_gather call (hard ucode limit)
WCH = 8         # chunks per gather call / A-build batch
PSW = 2         # PSUM tiles per scatter mega-window
GB_A = 6        # gather bufs, v2e stream A
GB_B = 3        # gather bufs, v2e stream B
GB_E = 8        # gather bufs, e2v stream

N_N, N_M = 50000, 20000
NS = N_N // NCORES               # 6250 nodes per core
NT_V = 49                        # local node tiles
NSP = NT_V * P                   # 6272 padded local nodes
NGP = NCORES * NSP               # 50176 global padded nodes
ES_OWN = N_M // NCORES           # 2500 edges per core
NT_EO = 20                       # owned edge tiles
MS_OWN = NT_EO * P               # 2560 padded owned edge rows
ME = NCORES * MS_OWN             # 20480 global padded edges
K_Y = 4                          # ytab AG chunks per layer
OWNR = MS_OWN // K_Y             # 640 local rows per ytab chunk
CH_E = ME // K_Y                 # 5120 global rows per ytab chunk
K_T = 7                          # table write groups (7 tiles each)
TCH_L = NSP // K_T               # 896 local rows per write group
TA_L = 4 * TCH_L                 # 3584 local rows in table half A
TB_L = NSP - TA_L                # 2688 local rows in table half B
SPLIT = NCORES * TA_L            # 28672 global rows in half A (< int16 max)
NGB = NCORES * TB_L              # 21504 global rows in half B
SHARED_AG = True                 # AllGather outputs in Shared address space


def _gid_edge(e):
    c = e // ES_OWN
    r = e - c * ES_OWN
    k = r // OWNR
    return k * CH_E + c * OWNR + (r - k * OWNR)


def _tgid_node(v):
    """Node-table row layout: two AllGather halves, block-major within each.

    half A = local rows [0, 3584) of each core -> global [c*3584 + u]
    half B = local rows [3584, 6272)          -> global SPLIT + [c*2688 + u']
    """
    c = v // NS
    u = v - c * NS
    return np.where(u < TA_L, c * TA_L + u, SPLIT + c * TB_L + (u - TA_L))


def _wrap_idx(flat):
    L = len(flat)
    assert L % 16 == 0
    blk = np.asarray(flat, np.int16).reshape(-1, 16).T.copy()
    return np.ascontiguousarray(np.tile(blk, (8, 1)))


def _build_stream(dst, src_idx, n_tiles, cpt):
    """Destination-sorted, per-tile 128-padded entry stream."""
    order = np.argsort(dst, kind="stable")
    dsts = np.asarray(dst)[order]
    srcs = np.asarray(src_idx)[order]
    tile_of = dsts // P
    counts = np.bincount(tile_of, minlength=n_tiles)
    base = np.concatenate([[0], np.cumsum(cpt * P)])
    L = int(base[-1])
    gidx = np.zeros(L, np.int64)
    ec = -np.ones(L, np.float32)
    starts = np.concatenate([[0], np.cumsum(counts)])
    off = np.arange(len(dsts)) - starts[tile_of]
    slot = base[tile_of] + off
    gidx[slot] = srcs
    ec[slot] = dsts - tile_of * P
    return gidx, ec


def _caps(raw, n_tiles, min1):
    cpt = None
    for dst, _ in raw:
        counts = np.bincount(np.asarray(dst) // P, minlength=n_tiles)
        c1 = (counts + P - 1) // P
        if min1:
            c1 = np.maximum(1, c1)
        cpt = c1 if cpt is None else np.maximum(cpt, c1)
    return cpt


def _pad_stream(g, ec, LP):
    gi = np.full(LP, -1, np.int64)
    gi[: len(g)] = g
    ecp = np.full(LP, -1.0, np.float32)
    ecp[: len(ec)] = ec
    ecb = ecp.astype(np.dtype("bfloat16"))
    return gi, np.ascontiguousarray(ecb.reshape(-1, P).T)


def _regs(L, LP):
    return [int(max(0, min(L - k * NI, NI))) for k in range(LP // NI)]


def _cols(arr, n_tiles):
    out = np.zeros((P, n_tiles), np.float32)
    a = np.asarray(arr, np.float32)
    for t in range(n_tiles):
        seg = a[t * P:(t + 1) * P]
        out[: len(seg), t] = seg
    return out


def _prep(inputs):
    V = np.asarray(inputs["V"]).astype(np.int64)
    E = np.asarray(inputs["E"]).astype(np.int64)
    X = np.asarray(inputs["X"], np.float32)
    S = np.asarray(inputs["S"], np.float32)
    bf = np.dtype("bfloat16")

    deg_v = np.bincount(V, minlength=N_N).astype(np.float64)
    cnt_e = np.bincount(E, minlength=N_M).astype(np.float64)
    deginv = np.where(deg_v > 0, 1.0 / np.maximum(deg_v, 1.0), 0.0)
    De = np.zeros(N_M, np.float64)
    np.add.at(De, E, deg_v[V])
    De = De / (cnt_e + 1.0)
    De_inv = np.where(De > 0, De ** -0.5, 1.0)
    coef_e = np.where(cnt_e > 0, De_inv / np.maximum(cnt_e, 1.0), 0.0)
    with np.errstate(divide="ignore"):
        Dv_inv = np.where(deg_v > 0, deg_v ** -0.5, 0.0)

    tg_all = _tgid_node(V)
    ge_all = _gid_edge(E)
    owner_e = E // ES_OWN
    r_e = E - owner_e * ES_OWN
    owner_v = V // NS
    u_v = V - owner_v * NS

    v2e_a_raw, v2e_b_raw, e2v_raw = [], [], []
    for c in range(NCORES):
        m = owner_e == c
        dst = r_e[m]
        src = tg_all[m]
        mA = src < SPLIT
        v2e_a_raw.append((dst[mA], src[mA]))
        v2e_b_raw.append((dst[~mA], src[~mA] - SPLIT))
        mv = owner_v == c
        e2v_raw.append((u_v[mv], ge_all[mv]))

    cpt_va = _caps(v2e_a_raw, NT_EO, min1=True)
    cpt_vb = _caps(v2e_b_raw, NT_EO, min1=False)
    cpt_e = _caps(e2v_raw, NT_V, min1=True)
    LvA = int(np.sum(cpt_va) * P)
    LvB = int(np.sum(cpt_vb) * P)
    Le = int(np.sum(cpt_e) * P)
    LvAP = ((LvA + NI - 1) // NI) * NI
    LvBP = max(NI, ((LvB + NI - 1) // NI) * NI)
    LeP = ((Le + NI - 1) // NI) * NI

    g = lambda k: np.asarray(inputs[k], np.float32)
    W = {}
    # layer 0
    Wv0, bv0, a0 = g("Wv0"), g("bv0"), g("a0")
    W["va0"] = np.ascontiguousarray((Wv0 @ a0)[:, None]).astype(bf)
    c0 = float(bv0 @ a0)
    W["Wv0t"] = np.ascontiguousarray(Wv0).astype(bf)
    W["bv0t"] = np.tile(bv0[None, :], (P, 1)).astype(np.float32)
    Wx0, bx0 = g("Wx0"), g("bx0")
    W["Wx0h0"] = np.ascontiguousarray(Wx0).astype(bf)
    W["bx0m1"] = np.tile((bx0 - 1.0)[None, :], (P, 1))
    # layer 1
    Wv1, bv1, a1 = g("Wv1"), g("bv1"), g("a1")
    va1 = Wv1 @ a1
    W["va1h0"] = np.ascontiguousarray(va1[:128, None]).astype(bf)
    W["va1h1"] = np.ascontiguousarray(va1[128:, None]).astype(bf)
    c1 = float(bv1 @ a1)
    W["Wv1h0"] = np.ascontiguousarray(Wv1[:128]).astype(bf)
    W["Wv1h1"] = np.ascontiguousarray(Wv1[128:]).astype(bf)
    W["bv1t"] = np.tile(bv1[None, :], (P, 1)).astype(np.float32)
    Wx1, bx1 = g("Wx1"), g("bx1")
    W["Wx1h0"] = np.ascontiguousarray(Wx1[:128]).astype(bf)
    W["Wx1h1"] = np.ascontiguousarray(Wx1[128:]).astype(bf)
    W["bx1m1"] = np.tile((bx1 - 1.0)[None, :], (P, 1))
    for l in range(2):
        Wt, bt = g(f"Wt{l}"), g(f"bt{l}")
        btf = bt - Wt[:256].sum(axis=0)
        W[f"Wt{l}h0"] = np.ascontiguousarray(Wt[:128]).astype(bf)
        W[f"Wt{l}h1"] = np.ascontiguousarray(Wt[128:256]).astype(bf)
        W[f"Wt{l}bot"] = np.ascontiguousarray(Wt[256:]).astype(bf)
        W[f"bt{l}f"] = np.tile(btf[None, :].astype(np.float32), (P, 1))
    Wf = g("Wf")
    W["Wfh0"] = np.ascontiguousarray(Wf[:128]).astype(bf)
    W["Wfh1"] = np.ascontiguousarray(Wf[128:]).astype(bf)
    W["bft"] = np.tile(g("bf")[None, :], (P, 1))

    iota = np.tile(np.arange(P, dtype=np.float32)[None, :], (P, 1))
    iota_rep = np.ascontiguousarray(
        np.broadcast_to(iota[:, None, :], (P, WCH, P))).astype(bf)

    # full X, block-major padded layout (col c*6272+u)
    XTg = np.zeros((128, NGP), np.float32)
    for c in range(NCORES):
        XTg[:, c * NSP:c * NSP + NS] = X[c * NS:(c + 1) * NS].T
    XTg = XTg.astype(bf)

    in_maps = []
    for c in range(NCORES):
        gva, ecva = _build_stream(*v2e_a_raw[c], NT_EO, cpt_va)
        gvb, ecvb = _build_stream(*v2e_b_raw[c], NT_EO, cpt_vb)
        ge_, ece = _build_stream(*e2v_raw[c], NT_V, cpt_e)
        gva_p, ecva_2d = _pad_stream(gva, ecva, LvAP)
        gvb_p, ecvb_2d = _pad_stream(gvb, ecvb, LvBP)
        ge_p, ece_2d = _pad_stream(ge_, ece, LeP)

        e0 = c * ES_OWN
        ST_own = np.zeros((MS_OWN, 64), np.float32)
        ST_own[:ES_OWN] = S[e0:e0 + ES_OWN]
        coef_own = np.zeros(MS_OWN, np.float32)
        coef_own[:ES_OWN] = coef_e[e0:e0 + ES_OWN]

        im = dict(
            XT=XTg,
            XTl=np.ascontiguousarray(XTg[:, c * NSP:(c + 1) * NSP]),
            ST=np.ascontiguousarray(ST_own.T).astype(bf),
            gva_idx=_wrap_idx(gva_p), gvb_idx=_wrap_idx(gvb_p),
            ge_idx=_wrap_idx(ge_p),
            ec_va=ecva_2d, ec_vb=ecvb_2d, ec_e=ece_2d,
            iota_rep=iota_rep,
            cvec=np.tile(np.array([[c0, c1, -1.0, 0.0]], np.float32),
                         (P, 1)),
            dgi=_cols(deginv[c * NS:(c + 1) * NS], NT_V),
            dvi=_cols(Dv_inv[c * NS:(c + 1) * NS], NT_V),
            cf=_cols(coef_own, NT_EO),
        )
        im.update(W)
        in_maps.append(im)

    meta = dict(cpt_va=[int(x) for x in cpt_va],
                cpt_vb=[int(x) for x in cpt_vb],
                cpt_e=[int(x) for x in cpt_e],
                LvAP=LvAP, LvBP=LvBP, LeP=LeP,
                regs_va=_regs(LvA, LvAP), regs_vb=_regs(LvB, LvBP),
                regs_e=_regs(Le, LeP), c0=c0, c1=c1)
    return in_maps, meta


# ---------------------------------------------------------------------------

def build_program(meta):
    nc = bacc.Bacc("TRN2", target_bir_lowering=False, debug=False,
                   num_devices=NCORES, num_swdge_queues=GQ)

    def din(name, shape, dt=F32):
        return nc.dram_tensor(name, shape, dt, kind="ExternalInput")

    XT = din("XT", [P, NGP], BF16)
    XTl = din("XTl", [P, NSP], BF16)
    ST = din("ST", [64, MS_OWN], BF16)
    gva_idx = din("gva_idx", [P, meta["LvAP"] // 16], I16)
    gvb_idx = din("gvb_idx", [P, meta["LvBP"] // 16], I16)
    ge_idx = din("ge_idx", [P, meta["LeP"] // 16], I16)
    nch_va = meta["LvAP"] // P
    nch_vb = meta["LvBP"] // P
    nch_e = meta["LeP"] // P
    ec_va = din("ec_va", [P, nch_va], BF16)
    ec_vb = din("ec_vb", [P, nch_vb], BF16)
    ec_e = din("ec_e", [P, nch_e], BF16)
    iota_rep = din("iota_rep", [P, WCH, P], BF16)
    cvec = din("cvec", [P, 4])
    dgi = din("dgi", [P, NT_V])
    dvi = din("dvi", [P, NT_V])
    cf = din("cf", [P, NT_EO])
    wshapes = dict(va0=[P, 1], Wv0t=[P, 256], Wx0h0=[P, 256],
                   va1h0=[P, 1], va1h1=[P, 1],
                   Wv1h0=[P, 256], Wv1h1=[P, 256],
                   Wx1h0=[P, 256], Wx1h1=[P, 256],
                   Wt0h0=[P, 256], Wt0h1=[P, 256], Wt0bot=[64, 256],
                   Wt1h0=[P, 256], Wt1h1=[P, 256], Wt1bot=[64, 256],
                   Wfh0=[P, 128], Wfh1=[P, 128])
    fshapes = dict(bv0t=[P, 256], bx0m1=[P, 256], bv1t=[P, 256],
                   bx1m1=[P, 256], bt0f=[P, 256], bt1f=[P, 256],
                   bft=[P, 128])
    Wd = {k: din(k, s, BF16) for k, s in wshapes.items()}
    Wd.update({k: din(k, s, F32) for k, s in fshapes.items()})

    yout = nc.dram_tensor("yout", [NS, 128], F32, kind="ExternalOutput")

    rg = [list(range(NCORES))]
    ag_space = "Shared" if SHARED_AG else "Local"

    with tile.TileContext(nc) as tc:
        ctx = ExitStack()
        sbuf = ctx.enter_context(tc.tile_pool(name="sbuf", bufs=2))
        psum = ctx.enter_context(tc.tile_pool(name="psum", bufs=2, space="PSUM"))
        dram = ctx.enter_context(tc.tile_pool(name="dram", bufs=1, space="DRAM"))
        cons = ctx.enter_context(tc.tile_pool(name="cons", bufs=1))

        iota_t = cons.tile([P, WCH, P], BF16, name="iota_t")
        nc.scalar.dma_start(iota_t[:], iota_rep[:])
        ident = cons.tile([P, P], F32, name="ident")
        make_identity(nc, ident[:])
        wt = {}
        for k, h in Wd.items():
            t_ = cons.tile(list(h.shape), h.dtype, name=f"w_{k}")
            nc.scalar.dma_start(t_[:], h[:])
            wt[k] = t_
        st_t = cons.tile([64, MS_OWN], BF16, name="st_t")
        nc.sync.dma_start(st_t[:], ST[:])
        ecva_t = cons.tile([P, nch_va], BF16, name="ecva_t")
        nc.scalar.dma_start(ecva_t[:], ec_va[:])
        ecvb_t = cons.tile([P, nch_vb], BF16, name="ecvb_t")
        nc.scalar.dma_start(ecvb_t[:], ec_vb[:])
        ece_t = cons.tile([P, nch_e], BF16, name="ece_t")
        nc.scalar.dma_start(ece_t[:], ec_e[:])
        gva_t = cons.tile([P, meta["LvAP"] // 16], I16, name="gva_t")
        nc.sync.dma_start(gva_t[:], gva_idx[:])
        gvb_t = cons.tile([P, meta["LvBP"] // 16], I16, name="gvb_t")
        nc.sync.dma_start(gvb_t[:], gvb_idx[:])
        ge_t = cons.tile([P, meta["LeP"] // 16], I16, name="ge_t")
        nc.sync.dma_start(ge_t[:], ge_idx[:])
        cvec_t = cons.tile([P, 4], F32, name="cvec_t")
        nc.scalar.dma_start(cvec_t[:], cvec[:])
        dgi_t = cons.tile([P, NT_V], F32, name="dgi_t")
        nc.scalar.dma_start(dgi_t[:], dgi[:])
        dgin_t = cons.tile([P, NT_V], F32, name="dgin_t")
        nc.vector.tensor_scalar_mul(out=dgin_t[:], in0=dgi_t[:], scalar1=-1.0)
        dvi_t = cons.tile([P, NT_V], F32, name="dvi_t")
        nc.scalar.dma_start(dvi_t[:], dvi[:])
        cf_t = cons.tile([P, NT_EO], F32, name="cf_t")
        nc.scalar.dma_start(cf_t[:], cf[:])

        # node-side x_init state lives in DRAM (SBUF is tight)
        sc_sb = cons.tile([P, NT_V], F32, name="sc_sb")
        xinit0_d = dram.tile([NSP, 256], BF16, name="xinit0_d")
        xinit1_d = dram.tile([NSP, 256], BF16, name="xinit1_d")

        # DRAM tables
        tab0 = dram.tile([NGP, 256], BF16, name="tab0")
        tab1l = dram.tile([NSP, 384], BF16, name="tab1l")
        tab1gA = dram.tile([SPLIT, 384], BF16, name="tab1gA",
                           addr_space=ag_space)
        tab1gB = dram.tile([NGB, 384], BF16, name="tab1gB",
                           addr_space=ag_space)
        tab2l = dram.tile([NSP, 128], BF16, name="tab2l")
        tab2gA = dram.tile([SPLIT, 128], BF16, name="tab2gA",
                           addr_space=ag_space)
        tab2gB = dram.tile([NGB, 128], BF16, name="tab2gB",
                           addr_space=ag_space)
        ytl = [dram.tile([MS_OWN, 256], BF16, name=f"ytl{l}") for l in range(2)]
        ytg = [dram.tile([ME, 256], BF16, name=f"ytg{l}", addr_space=ag_space)
               for l in range(2)]
        yt3l = dram.tile([MS_OWN, 128], BF16, name="yt3l")
        yt3g = dram.tile([ME, 128], BF16, name="yt3g", addr_space=ag_space)

        qctr = [0]

        def cc_ag(in_ap, out_ap):
            bass.BassGpSimd.collective_compute(
                nc.gpsimd, "AllGather", mybir.AluOpType.bypass,
                replica_groups=rg, ins=[in_ap], outs=[out_ap])

        def scatter_pass(streams, used_cols, n_tiles, on_tile):
            """Gather + one-hot-matmul segment sum over dest tiles."""
            S_ = len(streams)
            chunk_lists = []
            for st in streams:
                tof = []
                for t, n in enumerate(st["cpt"]):
                    tof += [t] * n
                chunk_lists.append(tof)
            order = []
            ks = [0] * S_
            for t in range(n_tiles):
                for s in range(S_):
                    for _ in range(streams[s]["cpt"][t]):
                        order.append((s, ks[s]))
                        ks[s] += 1
            first_c, last_c = {}, {}
            for pos, (s, k) in enumerate(order):
                t = chunk_lists[s][k]
                first_c.setdefault(t, pos)
                last_c[t] = pos
            g_tiles = [[None] * len(st["regs"]) for st in streams]
            emitted = [0] * S_

            def ensure_emitted(s, upto):
                st = streams[s]
                while emitted[s] <= min(upto, len(st["regs"]) - 1):
                    call = emitted[s]
                    if st["regs"][call] > 0:
                        gt = sbuf.tile([P, WCH, st["es"]], BF16,
                                       tag=st["ring"], bufs=st["gb"],
                                       name=f"g{st['tag']}_{call}")
                        nc.gpsimd.dma_gather(
                            out_ap=gt[:], in_ap=st["in_ap"],
                            idxs_ap=st["idx_t"][:, call * (NI // 16):
                                                (call + 1) * (NI // 16)],
                            num_idxs=NI, num_idxs_reg=st["regs"][call],
                            elem_size=st["es"], queue_num=qctr[0] % GQ)
                        qctr[0] += 1
                        g_tiles[s][call] = gt
                    emitted[s] += 1

            a_cur = [[None, -1] for _ in range(S_)]
            mega = [None, -1]
            for pos, (s, k) in enumerate(order):
                st = streams[s]
                t = chunk_lists[s][k]
                call, j = k // WCH, k % WCH
                ensure_emitted(s, call + st["gb"] - 1)
                gt = g_tiles[s][call]
                if gt is None:
                    continue
                w = k // WCH
                if a_cur[s][1] != w:
                    ab = sbuf.tile([P, WCH, P], BF16, tag=f"A{s}", bufs=2,
                                   name=f"A{st['tag']}_{w}")
                    nc.vector.tensor_tensor(
                        out=ab[:],
                        in0=st["ec_t"][:, w * WCH:(w + 1) * WCH].to_broadcast(
                            [P, WCH, P]),
                        in1=iota_t[:],
                        op=mybir.AluOpType.is_equal)
                    a_cur[s] = [ab, w]
                mw = t // PSW
                if mega[1] != mw:
                    mega = [psum.tile([P, PSW, 512], F32, tag="ps", bufs=2,
                                      name=f"ps{st['tag']}_{mw}"), mw]
                pt = mega[0]
                q = t % PSW
                nc.tensor.matmul(
                    out=pt[:, q, 0:used_cols],
                    lhsT=a_cur[s][0][:, j, :],
                    rhs=gt[:, j, 0:used_cols],
                    start=(pos == first_c[t]), stop=(pos == last_c[t]))
                if pos == last_c[t]:
                    on_tile(t, pt, q)

        def elu_u(z_ap, w, cols, tag, i):
            """relu(z) + exp(min(z,0)) = elu(z) + 1 (2 DVE + 2 ACT ops)."""
            mn = sbuf.tile([P, cols], F32, tag="mn", bufs=2, name=f"mn{tag}{i}")
            nc.vector.tensor_scalar_min(out=mn[:w], in0=z_ap, scalar1=0.0)
            ex = sbuf.tile([P, cols], F32, tag="ex", bufs=2, name=f"ex{tag}{i}")
            nc.scalar.activation(ex[:w], mn[:w], AF.Exp)
            rl = sbuf.tile([P, cols], F32, tag="rl", bufs=2, name=f"rl{tag}{i}")
            nc.scalar.activation(rl[:w], z_ap, AF.Relu)
            u = sbuf.tile([P, cols], F32, tag="u", bufs=2, name=f"u{tag}{i}")
            nc.vector.tensor_add(u[:w], rl[:w], ex[:w])
            return u

        def elu_u_psum(pt_ap, scale_ap, nscale_ap, tag, i):
            """elu(psum*scale) + 1 from PSUM: 3 ACT + 1 DVE, no z staging."""
            rl = sbuf.tile([P, 256], F32, tag="rl", bufs=2, name=f"rl{tag}{i}")
            nc.scalar.activation(rl[:], pt_ap, AF.Relu, scale=scale_ap)
            r2 = sbuf.tile([P, 256], F32, tag="r2", bufs=2, name=f"r2{tag}{i}")
            nc.scalar.activation(r2[:], pt_ap, AF.Relu, scale=nscale_ap)
            ex = sbuf.tile([P, 256], F32, tag="ex", bufs=2, name=f"ex{tag}{i}")
            nc.scalar.activation(ex[:], r2[:], AF.Exp,
                                 scale=cvec_t[:, 2:3])
            u = sbuf.tile([P, 256], F32, tag="u", bufs=2, name=f"u{tag}{i}")
            nc.vector.tensor_add(u[:], rl[:], ex[:])
            return u

        def transpose_pair(src_ap, tag, i):
            """[P,256] f32 -> two [P,P] bf16 transposed tiles (copies on ACT)."""
            pT = psum.tile([P, 512], F32, tag="pT", bufs=2,
                           name=f"pT{tag}_{i}")
            outs = []
            for hi in range(2):
                nc.tensor.transpose(out=pT[:, hi * P:(hi + 1) * P],
                                    in_=src_ap[:, hi * P:(hi + 1) * P],
                                    identity=ident[:, :])
                sT = sbuf.tile([P, P], BF16, tag="sT", bufs=4,
                               name=f"sT{tag}_{i}_{hi}")
                nc.scalar.activation(sT[:], pT[:, hi * P:(hi + 1) * P],
                                     AF.Copy)
                outs.append(sT)
            return outs

        def wt_matmuls(l, t, uT, tag):
            py = psum.tile([P, 512], F32, tag="pd", bufs=2, name=f"py{tag}_{t}")
            nc.tensor.matmul(out=py[:, 0:256],
                             lhsT=st_t[:, t * P:(t + 1) * P],
                             rhs=wt[f"Wt{l}bot"][:], start=True, stop=False)
            nc.tensor.matmul(out=py[:, 0:256], lhsT=uT[0][:],
                             rhs=wt[f"Wt{l}h0"][:], start=False, stop=False)
            nc.tensor.matmul(out=py[:, 0:256], lhsT=uT[1][:],
                             rhs=wt[f"Wt{l}h1"][:], start=False, stop=True)
            return py

        # ------------------------------------------------------------------
        # stage 1: build full layer-0 table [ew*X | ew] + local xinit0
        ident_bf = cons.tile([P, P], BF16, name="ident_bf")
        nc.vector.tensor_copy(out=ident_bf[:], in_=ident[:])
        for c8 in range(NCORES):
            for kk in range(K_T):
                ps7 = psum.tile([P, 512], F32, tag="pd", bufs=2,
                                name=f"ps7_{c8}_{kk}")
                col0 = c8 * NSP + kk * TCH_L
                xtg = sbuf.tile([P, TCH_L], BF16, tag="xtg", bufs=3,
                                name=f"xtg_{c8}_{kk}")
                nc.sync.dma_start(xtg[:], XT[:, col0:col0 + TCH_L])
                for j in range(K_T):
                    nc.tensor.matmul(out=ps7[:, j:j + 1],
                                     lhsT=xtg[:, j * P:(j + 1) * P],
                                     rhs=wt["va0"][:], start=True, stop=True)
                sc7 = sbuf.tile([P, K_T], F32, tag="sc7", bufs=2,
                                name=f"sc7_{c8}_{kk}")
                nc.vector.tensor_scalar_add(out=sc7[:], in0=ps7[:, 0:K_T],
                                            scalar1=meta["c0"])
                lr7 = sbuf.tile([P, K_T], F32, tag="lr7", bufs=2,
                                name=f"lr7_{c8}_{kk}")
                nc.vector.tensor_scalar_mul(out=lr7[:], in0=sc7[:],
                                            scalar1=NEG_SLOPE)
                mx7 = sbuf.tile([P, K_T], F32, tag="mx7", bufs=2,
                                name=f"mx7_{c8}_{kk}")
                nc.vector.tensor_tensor(out=mx7[:], in0=sc7[:], in1=lr7[:],
                                        op=mybir.AluOpType.max)
                ew7 = sbuf.tile([P, K_T], F32, tag="ew7", bufs=2,
                                name=f"ew7_{c8}_{kk}")
                nc.scalar.activation(ew7[:], mx7[:], AF.Exp)
                stg = sbuf.tile([P, K_T, 256], BF16, tag="stg0", bufs=3,
                                name=f"stg0_{c8}_{kk}")
                for j0, nj in ((0, 4), (4, 3)):
                    pTq = psum.tile([P, 512], BF16, tag="pT", bufs=2,
                                    name=f"xpT_{c8}_{kk}_{j0}")
                    for j in range(nj):
                        nc.tensor.transpose(
                            out=pTq[:, j * P:(j + 1) * P],
                            in_=xtg[:, (j0 + j) * P:(j0 + j + 1) * P],
                            identity=ident_bf[:, :])
                    nc.vector.tensor_tensor(
                        out=stg[:, j0:j0 + nj, 0:128],
                        in0=pTq[:, 0:nj * P].rearrange("p (j c) -> p j c", j=nj),
                        in1=ew7[:, j0:j0 + nj].to_broadcast([P, nj, P]),
                        op=mybir.AluOpType.mult)
                nc.vector.tensor_copy(out=stg[:, :, 128:129],
                                      in_=ew7[:].to_broadcast([P, K_T, 1]))
                if kk < 4:
                    r0 = c8 * TA_L + kk * TCH_L
                else:
                    r0 = SPLIT + c8 * TB_L + (kk - 4) * TCH_L
                nc.sync.dma_start(
                    out=tab0[r0:r0 + TCH_L, :].rearrange(
                        "(j p) c -> p j c", p=P),
                    in_=stg[:])
        for kk in range(K_T):
            xtg = sbuf.tile([P, TCH_L], BF16, tag="xtg", bufs=3,
                            name=f"xtlg_{kk}")
            nc.sync.dma_start(xtg[:], XTl[:, kk * TCH_L:(kk + 1) * TCH_L])
            for j in range(K_T):
                t = kk * K_T + j
                pi = psum.tile([P, 512], F32, tag="pd", bufs=2,
                               name=f"pi0_{t}")
                nc.tensor.matmul(out=pi[:, 0:256],
                                 lhsT=xtg[:, j * P:(j + 1) * P],
                                 rhs=wt["Wx0h0"][:], start=True, stop=True)
                xi0 = sbuf.tile([P, 256], BF16, tag="xi", bufs=2,
                                name=f"xi0_{t}")
                nc.vector.tensor_add(xi0[:], pi[:, 0:256], wt["bx0m1"][:])
                nc.sync.dma_start(out=xinit0_d[t * P:(t + 1) * P, :],
                                  in_=xi0[:])

        # ------------------------------------------------------------------
        # stage 2: L0 v2e scatter into owned edges + fused edge epilogue
        def v2e0_tile(t, pt, q):
            dc = sbuf.tile([P, 1], F32, tag="dc", bufs=2, name=f"dc0_{t}")
            nc.vector.tensor_scalar_max(out=dc[:], in0=pt[:, q, 128:129],
                                        scalar1=1e-35)
            di = sbuf.tile([P, 1], F32, tag="di", bufs=2, name=f"di0_{t}")
            nc.vector.reciprocal(di[:], dc[:])
            zx = sbuf.tile([P, P], F32, tag="zx", bufs=2, name=f"zx0_{t}")
            nc.scalar.activation(zx[:], pt[:, q, 0:128], AF.Copy,
                                 scale=di[:, :])
            pT = psum.tile([P, 512], F32, tag="pT", bufs=2, name=f"zxT_{t}")
            nc.tensor.transpose(out=pT[:, 0:P], in_=zx[:], identity=ident[:, :])
            zxT = sbuf.tile([P, P], BF16, tag="sT", bufs=4, name=f"zxTs_{t}")
            nc.scalar.activation(zxT[:], pT[:, 0:P], AF.Copy)
            pz = psum.tile([P, 512], F32, tag="pd", bufs=2, name=f"pz0_{t}")
            nc.tensor.matmul(out=pz[:, 0:256], lhsT=zxT[:], rhs=wt["Wv0t"][:],
                             start=True, stop=True)
            F = sbuf.tile([P, 256], F32, tag="F", bufs=2, name=f"F0_{t}")
            nc.vector.tensor_add(F[:], pz[:, 0:256], wt["bv0t"][:])
            u = elu_u(F[:], P, 256, "e0", t)
            uT = transpose_pair(u[:], "u0", t)
            py = wt_matmuls(0, t, uT, "y0")
            yt = sbuf.tile([P, 256], BF16, tag="yt", bufs=2, name=f"yt0_{t}")
            nc.vector.tensor_add(yt[:], py[:, 0:256], wt["bt0f"][:])
            nc.sync.dma_start(out=ytl[0][t * P:(t + 1) * P, :], in_=yt[:])
            if t % 5 == 4:
                k = t // 5
                cc_ag(ytl[0][k * OWNR:(k + 1) * OWNR, :],
                      ytg[0][k * CH_E:(k + 1) * CH_E, :])

        scatter_pass(
            [dict(in_ap=tab0[0:SPLIT, :], es=256, idx_t=gva_t, ec_t=ecva_t,
                  cpt=meta["cpt_va"], regs=meta["regs_va"], tag="va0",
                  gb=GB_A, ring="gva"),
             dict(in_ap=tab0[SPLIT:NGP, :], es=256, idx_t=gvb_t, ec_t=ecvb_t,
                  cpt=meta["cpt_vb"], regs=meta["regs_vb"], tag="vb0",
                  gb=GB_B, ring="gvb")],
            129, NT_EO, v2e0_tile)

        # ------------------------------------------------------------------
        # stage 3: L0 e2v + h1/hT/xinit1 + local L1 table + AG
        h_tiles = {}

        def flush_tab1(t):
            """Premultiply h tiles of the finished 7-tile group, write+AG."""
            nj = t % K_T + 1
            kk = t // K_T
            t0 = t - nj + 1
            sc7 = sbuf.tile([P, K_T], F32, tag="sc7", bufs=2, name=f"sc1_{kk}")
            nc.vector.tensor_scalar_add(out=sc7[:, 0:nj],
                                        in0=sc_sb[:, t0:t + 1],
                                        scalar1=meta["c1"])
            lr7 = sbuf.tile([P, K_T], F32, tag="lr7", bufs=2, name=f"lr1_{kk}")
            nc.vector.tensor_scalar_mul(out=lr7[:, 0:nj], in0=sc7[:, 0:nj],
                                        scalar1=NEG_SLOPE)
            mx7 = sbuf.tile([P, K_T], F32, tag="mx7", bufs=2, name=f"mx1_{kk}")
            nc.vector.tensor_tensor(out=mx7[:, 0:nj], in0=sc7[:, 0:nj],
                                    in1=lr7[:, 0:nj], op=mybir.AluOpType.max)
            ew7 = sbuf.tile([P, K_T], F32, tag="ew7", bufs=2, name=f"ew1_{kk}")
            nc.scalar.activation(ew7[:, 0:nj], mx7[:, 0:nj], AF.Exp)
            stg = sbuf.tile([P, K_T, 384], BF16, tag="stg1", bufs=2,
                            name=f"stg1_{kk}")
            for jj in range(nj):
                tt = t0 + jj
                h = h_tiles.pop(tt)
                nc.vector.tensor_scalar_mul(out=stg[:, jj, 0:256], in0=h[:],
                                            scalar1=ew7[:, jj:jj + 1])
            nc.vector.tensor_copy(
                out=stg[:, 0:nj, 256:257],
                in_=ew7[:, 0:nj].to_broadcast([P, nj, 1]))
            r0 = kk * TCH_L
            nc.sync.dma_start(
                out=tab1l[r0:r0 + nj * P, :].rearrange("(j p) c -> p j c", p=P),
                in_=stg[:, 0:nj, :])
            if kk == 3:
                cc_ag(tab1l[0:TA_L, :], tab1gA[:])
            elif kk == K_T - 1:
                cc_ag(tab1l[TA_L:NSP, :], tab1gB[:])

        def e2v0_tile(t, pt, q):
            xi0 = sbuf.tile([P, 256], BF16, tag="xil", bufs=3, name=f"xi0l_{t}")
            nc.scalar.dma_start(xi0[:], xinit0_d[t * P:(t + 1) * P, :])
            u = elu_u_psum(pt[:, q, 0:256], dgi_t[:, t:t + 1],
                           dgin_t[:, t:t + 1], "n0", t)
            h = sbuf.tile([P, 256], F32, tag="h", bufs=8, name=f"h1_{t}")
            nc.vector.tensor_add(h[:], u[:], xi0[:])
            h_tiles[t] = h
            hTt = transpose_pair(h[:], "h1", t)
            pi = psum.tile([P, 512], F32, tag="pd", bufs=2, name=f"pi1_{t}")
            nc.tensor.matmul(out=pi[:, 0:256], lhsT=hTt[0][:],
                             rhs=wt["Wx1h0"][:], start=True, stop=False)
            nc.tensor.matmul(out=pi[:, 0:256], lhsT=hTt[1][:],
                             rhs=wt["Wx1h1"][:], start=False, stop=True)
            xi1 = sbuf.tile([P, 256], BF16, tag="xi", bufs=2, name=f"xi1_{t}")
            nc.vector.tensor_add(xi1[:], pi[:, 0:256], wt["bx1m1"][:])
            nc.sync.dma_start(out=xinit1_d[t * P:(t + 1) * P, :], in_=xi1[:])
            nc.tensor.matmul(out=pi[:, 256:257], lhsT=hTt[0][:],
                             rhs=wt["va1h0"][:], start=True, stop=False)
            nc.tensor.matmul(out=pi[:, 256:257], lhsT=hTt[1][:],
                             rhs=wt["va1h1"][:], start=False, stop=True)
            nc.vector.tensor_copy(out=sc_sb[:, t:t + 1], in_=pi[:, 256:257])
            if t % K_T == K_T - 1:
                flush_tab1(t)

        scatter_pass(
            [dict(in_ap=ytg[0][:], es=256, idx_t=ge_t, ec_t=ece_t,
                  cpt=meta["cpt_e"], regs=meta["regs_e"], tag="e0",
                  gb=GB_E, ring="ge")],
            256, NT_V, e2v0_tile)

        # ------------------------------------------------------------------
        # stage 4: L1 v2e + fused edge epilogue
        def v2e1_tile(t, pt, q):
            dc = sbuf.tile([P, 1], F32, tag="dc", bufs=2, name=f"dc1_{t}")
            nc.vector.tensor_scalar_max(out=dc[:], in0=pt[:, q, 256:257],
                                        scalar1=1e-35)
            di = sbuf.tile([P, 1], F32, tag="di", bufs=2, name=f"di1_{t}")
            nc.vector.reciprocal(di[:], dc[:])
            zd = sbuf.tile([P, 256], F32, tag="zd", bufs=2, name=f"zd1_{t}")
            nc.scalar.activation(zd[:], pt[:, q, 0:256], AF.Copy,
                                 scale=di[:, :])
            zT = transpose_pair(zd[:], "z1", t)
            pz = psum.tile([P, 512], F32, tag="pd", bufs=2, name=f"pz1_{t}")
            nc.tensor.matmul(out=pz[:, 0:256], lhsT=zT[0][:],
                             rhs=wt["Wv1h0"][:], start=True, stop=False)
            nc.tensor.matmul(out=pz[:, 0:256], lhsT=zT[1][:],
                             rhs=wt["Wv1h1"][:], start=False, stop=True)
            F = sbuf.tile([P, 256], F32, tag="F", bufs=2, name=f"F1_{t}")
            nc.vector.tensor_add(F[:], pz[:, 0:256], wt["bv1t"][:])
            u = elu_u(F[:], P, 256, "e1", t)
            uT = transpose_pair(u[:], "u1", t)
            py = wt_matmuls(1, t, uT, "y1")
            yt = sbuf.tile([P, 256], BF16, tag="yt", bufs=2, name=f"yt1_{t}")
            nc.vector.tensor_add(yt[:], py[:, 0:256], wt["bt1f"][:])
            nc.sync.dma_start(out=ytl[1][t * P:(t + 1) * P, :], in_=yt[:])
            if t % 5 == 4:
                k = t // 5
                cc_ag(ytl[1][k * OWNR:(k + 1) * OWNR, :],
                      ytg[1][k * CH_E:(k + 1) * CH_E, :])

        scatter_pass(
            [dict(in_ap=tab1g[0:SPLIT, :], es=384, idx_t=gva_t, ec_t=ecva_t,
                  cpt=meta["cpt_va"], regs=meta["regs_va"], tag="va1",
                  gb=GB_A, ring="gva1"),
             dict(in_ap=tab1g[SPLIT:NGP, :], es=384, idx_t=gvb_t, ec_t=ecvb_t,
                  cpt=meta["cpt_vb"], regs=meta["regs_vb"], tag="vb1",
                  gb=GB_B, ring="gvb1")],
            257, NT_EO, v2e1_tile)

        # ------------------------------------------------------------------
        # stage 5: L1 e2v -> h2 -> local T2 table + AG
        stg2 = [None]

        def e2v1_tile(t, pt, q):
            xi1 = sbuf.tile([P, 256], BF16, tag="xil", bufs=3, name=f"xi1l_{t}")
            nc.scalar.dma_start(xi1[:], xinit1_d[t * P:(t + 1) * P, :])
            u = elu_u_psum(pt[:, q, 0:256], dgi_t[:, t:t + 1],
                           dgin_t[:, t:t + 1], "n1", t)
            h2 = sbuf.tile([P, 256], F32, tag="h", bufs=8, name=f"h2_{t}")
            nc.vector.tensor_add(h2[:], u[:], xi1[:])
            h2T = transpose_pair(h2[:], "h2", t)
            pf = psum.tile([P, 512], F32, tag="pd", bufs=2, name=f"pf2_{t}")
            nc.tensor.matmul(out=pf[:, 0:128], lhsT=h2T[0][:],
                             rhs=wt["Wfh0"][:], start=True, stop=False)
            nc.tensor.matmul(out=pf[:, 0:128], lhsT=h2T[1][:],
                             rhs=wt["Wfh1"][:], start=False, stop=True)
            if stg2[0] is None:
                stg2[0] = sbuf.tile([P, K_T, 128], BF16, tag="stg2", bufs=2,
                                    name=f"stg2_{t}")
            jj = t % K_T
            nc.vector.tensor_add(stg2[0][:, jj, :], pf[:, 0:128], wt["bft"][:])
            if jj == K_T - 1:
                kk = t // K_T
                r0 = kk * TCH_L
                nc.sync.dma_start(
                    out=tab2l[r0:r0 + TCH_L, :].rearrange(
                        "(j p) c -> p j c", p=P),
                    in_=stg2[0][:])
                stg2[0] = None
                if kk == 3:
                    cc_ag(tab2l[0:TA_L, :], tab2gA[:])
                elif kk == K_T - 1:
                    cc_ag(tab2l[TA_L:NSP, :], tab2gB[:])

        scatter_pass(
            [dict(in_ap=ytg[1][:], es=256, idx_t=ge_t, ec_t=ece_t,
                  cpt=meta["cpt_e"], regs=meta["regs_e"], tag="e1",
                  gb=GB_E, ring="ge")],
            256, NT_V, e2v1_tile)

        # ------------------------------------------------------------------
        # stage 6: HC v2e (linear: sum T2 rows, scale by coef)
        def v2e2_tile(t, pt, q):
            yt = sbuf.tile([P, 128], BF16, tag="yt3", bufs=2, name=f"yt3_{t}")
            nc.vector.tensor_scalar_mul(out=yt[:], in0=pt[:, q, 0:128],
                                        scalar1=cf_t[:, t:t + 1])
            nc.sync.dma_start(out=yt3l[t * P:(t + 1) * P, :], in_=yt[:])
            if t % 5 == 4:
                k = t // 5
                cc_ag(yt3l[k * OWNR:(k + 1) * OWNR, :],
                      yt3g[k * CH_E:(k + 1) * CH_E, :])

        scatter_pass(
            [dict(in_ap=tab2g[0:SPLIT, :], es=128, idx_t=gva_t, ec_t=ecva_t,
                  cpt=meta["cpt_va"], regs=meta["regs_va"], tag="va2",
                  gb=GB_A, ring="gva2"),
             dict(in_ap=tab2g[SPLIT:NGP, :], es=128, idx_t=gvb_t, ec_t=ecvb_t,
                  cpt=meta["cpt_vb"], regs=meta["regs_vb"], tag="vb2",
                  gb=GB_B, ring="gvb2")],
            128, NT_EO, v2e2_tile)

        # ------------------------------------------------------------------
        # stage 7: HC e2v -> yout
        def e2v2_tile(t, pt, q):
            w = min(P, NS - t * P)
            ot = sbuf.tile([P, 128], F32, tag="fo", bufs=2, name=f"fo_{t}")
            nc.vector.tensor_scalar_mul(out=ot[:w], in0=pt[:w, q, 0:128],
                                        scalar1=dvi_t[:w, t:t + 1])
            nc.sync.dma_start(out=yout[t * P:t * P + w, :], in_=ot[:w])

        scatter_pass(
            [dict(in_ap=yt3g[:], es=128, idx_t=ge_t, ec_t=ece_t,
                  cpt=meta["cpt_e"], regs=meta["regs_e"], tag="e2",
                  gb=GB_E, ring="ge")],
            128, NT_V, e2v2_tile)
        ctx.close()

    nc.compile()
    return nc


_CACHED = {}


def kernel(**inputs):
    in_maps, meta = _prep(inputs)
    key = (meta["LvAP"], meta["LvBP"], meta["LeP"], tuple(meta["cpt_va"]),
           tuple(meta["cpt_vb"]), tuple(meta["cpt_e"]), meta["c0"], meta["c1"])
    if key not in _CACHED:
        _CACHED[key] = build_program(meta)
    nc = _CACHED[key]
    res = run_bass_kernel_spmd(nc, in_maps, list(range(NCORES)))
    out = np.concatenate([res.results[c]["yout"] for c in range(NCORES)],
                         axis=0)
    return np.ascontiguousarray(out.astype(np.float32))
